# revision 25
# baseline (speedup 1.0000x reference)
"""YOLOv3-style detection decode kernel for Trainium2 (8 NeuronCores).

Contract: kernel(**inputs) takes the FULL unsharded inputs (as produced by
setup_inputs) and returns (boxes [N,6] f32, mask [N] bool) matching the
reference. Internally the batch (32) is sharded 8 ways (4 per core), one
SPMD Bass/Tile program runs on all 8 cores with per-core input maps, and
the host reassembles the full outputs.

Per-core pipeline (per scale):
  - contiguous channel-major DMA loads: class slabs [80, HW] per (b, anchor),
    field channels into a batched [72, HW] tile (rows f*12 + (a*4+b))
  - field math at full vector width: objectness mask, sigmoid(p),
    (x+gx)*scale, (y+gy)*scale, exp(w+ln(aw/case)), exp(h+ln(ah/case))
  - one permute DMA into transpose-ready layout (rows b*21 + a*6 + f)
  - PE data-as-weights transposes [80,128] -> [128,80] into PSUM (6/bank),
    DVE segmented reduce_max, ACT copy PSUM->SBUF, DVE max_index
    (exact first-occurrence argmax, matching jnp.argmax)
  - PE transposes of the 21-row field blocks to candidate-major, cls merged
    as uint32->f32 copy, contiguous DMA of boxes (18 f32/candidate-chunk row)
    and mask
"""

import threading
from contextlib import ExitStack

import numpy as np

import concourse.bacc as bacc
import concourse.bass as bass
import concourse.mybir as mybir
import concourse.tile as tile
from concourse.bass_utils import run_bass_kernel_spmd
from concourse.tile import add_dep_helper


def _dep(frm, *tos):
    """Explicit dependency edges: raw (multi-dim-partition) APs are tracked
    at last-writer granularity only, and DMA->DMA WAW deps do not order
    completion -- so readers must depend on every DMA writer explicitly."""
    f = getattr(frm, "ins", frm)
    for t in tos:
        add_dep_helper(f, getattr(t, "ins", t), reason="raw-ap-dep")

F32 = mybir.dt.float32
U32 = mybir.dt.uint32

SCALES = [(13, 32.0), (26, 16.0), (52, 8.0)]
B_LOCAL = 4  # batches per core (32 / 8)
N_CORES = 8
NCLS = 80

HWS = [s * s for s, _ in SCALES]              # 169, 676, 2704
LOCAL_BASES = []
_acc = 0
for _hw in HWS:
    LOCAL_BASES.append(_acc)
    _acc += B_LOCAL * _hw * 3
N_LOCAL = _acc                                 # 42588
N_FULL = N_LOCAL * N_CORES                     # 340704

PAD_VAL = -1.0e30


def _ap(t, offset, ap):
    return bass.AP(tensor=t.tensor if isinstance(t, bass.AP) else t,
                   offset=offset, ap=ap)


def _build_program():
    nc = bacc.Bacc()

    xs = {}
    for (s, _t), hw in zip(SCALES, HWS):
        xs[s] = nc.dram_tensor(f"x{s}", [B_LOCAL, 255, hw], F32,
                               kind="ExternalInput")
    nblob = 156 + 24 * sum(HWS)
    cblob = nc.dram_tensor("cblob", [nblob], F32, kind="ExternalInput")
    ident = nc.dram_tensor("ident", [128, 128], F32, kind="ExternalInput")

    boxes_o = nc.dram_tensor("boxes", [N_LOCAL, 6], F32, kind="ExternalOutput")
    mask_o = nc.dram_tensor("maskf", [N_LOCAL], F32, kind="ExternalOutput")

    # Raw (multi-dim-partition) access patterns are not precisely tracked by
    # Tile's subtile dependency machinery; force whole-tile granularity so
    # every reader conservatively waits for all prior writers of a tile.
    import os
    prev = os.environ.get("BY_DEFAULT_DISABLE_SUBTILE_DEPS")
    os.environ["BY_DEFAULT_DISABLE_SUBTILE_DEPS"] = "1"
    try:
        with tile.TileContext(nc) as tc:
            with ExitStack() as ctx:
                _emit(ctx, tc, nc, xs, cblob, ident, boxes_o, mask_o)
    finally:
        if prev is None:
            os.environ.pop("BY_DEFAULT_DISABLE_SUBTILE_DEPS", None)
        else:
            os.environ["BY_DEFAULT_DISABLE_SUBTILE_DEPS"] = prev
    nc.compile()
    return nc


def _emit(ctx, tc, nc, xs, cblob, ident, boxes_o, mask_o):
    consts = ctx.enter_context(tc.tile_pool(name="consts", bufs=1))
    fpool = ctx.enter_context(tc.tile_pool(name="fpool", bufs=1))
    cpool = ctx.enter_context(tc.tile_pool(name="cpool", bufs=6))
    spool = ctx.enter_context(tc.tile_pool(name="spool", bufs=3))
    mpool = ctx.enter_context(tc.tile_pool(name="mpool", bufs=3))
    ipool = ctx.enter_context(tc.tile_pool(name="ipool", bufs=5))
    opool = ctx.enter_context(tc.tile_pool(name="opool", bufs=2))
    pc = ctx.enter_context(tc.tile_pool(name="pc", bufs=4, space="PSUM"))
    po = ctx.enter_context(tc.tile_pool(name="po", bufs=2, space="PSUM"))

    # --- constants ---------------------------------------------------------
    id_sb = consts.tile([128, 128], F32, tag="id", name="id_sb")
    nc.sync.dma_start(out=id_sb, in_=ident[:, :])

    # consts blob (host pre-replicated): one simple DMA per tile
    pr = cblob[:]
    thresh_rep = consts.tile([12, 1], F32, tag="thresh", name="thresh_rep")
    nc.sync.dma_start(out=thresh_rep, in_=pr[0:12])
    scale_rep, lnwh = {}, {}
    goff = 156
    grid_off = {}
    for i, (s, _t) in enumerate(SCALES):
        # scale replicated at partition bases 32 and 64 (for x and y ops)
        scl = consts.tile([76, 1], F32, tag=f"scl{s}", name=f"scl{s}")
        d_scl = nc.sync.dma_start(
            out=_ap(scl, 32, [[1, 12], [1, 1]]),
            in_=pr[12 + i * 24: 12 + i * 24 + 12])
        d_scl64 = nc.sync.dma_start(
            out=_ap(scl, 64, [[1, 12], [1, 1]]),
            in_=pr[12 + i * 24 + 12: 12 + i * 24 + 24])
        scale_rep[s] = scl
        scale_rep[(s, "w32")] = d_scl
        scale_rep[(s, "w64")] = d_scl64
        # rows 0..11: ln(aw/case) by partition ba=a*4+b; rows 12..23: ln(ah)
        lw = consts.tile([24, 1], F32, tag=f"lnwh{s}", name=f"lnwh{s}")
        d_lnw = nc.sync.dma_start(out=lw,
                                  in_=pr[84 + i * 24: 84 + i * 24 + 24])
        lnwh[(s, "w")] = d_lnw
        lnwh[s] = lw
        grid_off[s] = goff
        goff += 24 * HWS[i]

    # --- static shared tiles (single instance for the whole program; no
    # slot recycling, since raw-AP accesses are not release-tracked) ------
    HWMAX = max(HWS)
    CPADW_MAX = ((HWMAX + 127) // 128) * 128
    # T1: obj rows 0..11, x rows 32..43, y rows 64..75 (row = base + ba)
    T1 = fpool.tile([76, HWMAX], F32, tag="T1", name="T1")
    # T2: w rows 0..11, h rows 12..23
    T2 = fpool.tile([24, HWMAX], F32, tag="T2", name="T2")
    # M: mask rows 0..11
    M = fpool.tile([12, HWMAX], F32, tag="M", name="M")
    # grid constants replicated x12: gx rows 32..43, gy rows 64..75
    Gp = fpool.tile([76, HWMAX], F32, tag="Gp", name="Gp")
    # transpose-ready per-b tiles: rows a*6 + f (f=5 cls placeholder),
    # rows 18 + a -> mask (base partition 0 for PE matmul)
    Gt = []
    gt_ms = []
    t1_ms = nc.gpsimd.memset(T1, 0.0)
    t2_ms = nc.gpsimd.memset(T2, 0.0)
    nc.gpsimd.memset(M, 0.0)
    nc.gpsimd.memset(Gp, 0.0)
    for b in range(B_LOCAL):
        Gtb = fpool.tile([21, CPADW_MAX], F32, tag=f"Gt{b}", name=f"Gt{b}")
        gt_ms.append(nc.gpsimd.memset(Gtb, 0.0))
        Gt.append(Gtb)

    for i, ((s, t), hw) in enumerate(zip(SCALES, HWS)):
        if i > 0:
            # full barrier between scales: the static tiles above are
            # reused across scales with raw-AP accesses
            tc.strict_bb_all_engine_barrier()
        nch = (hw + 127) // 128          # candidate chunks of 128
        cpadw = nch * 128
        x = xs[s][:, :, :]               # [4, 255, hw]

        d_gp32 = nc.sync.dma_start(
            out=_ap(Gp, 32 * HWMAX, [[HWMAX, 12], [1, hw]]),
            in_=_ap(pr, grid_off[s], [[hw, 12], [1, hw]]))
        d_gp64 = nc.sync.dma_start(
            out=_ap(Gp, 64 * HWMAX, [[HWMAX, 12], [1, hw]]),
            in_=_ap(pr, grid_off[s] + 12 * hw, [[hw, 12], [1, hw]]))

        # --- loads ---------------------------------------------------------
        C = {}
        c_ms = {}
        for b in range(B_LOCAL):
            for a in range(3):
                ba = a * 4 + b
                Cb = cpool.tile([80, cpadw], F32, tag="C", name="Cb")
                C[(b, a)] = Cb
                if cpadw > hw:
                    c_ms[(b, a)] = nc.gpsimd.memset(Cb[:, hw:], PAD_VAL)
                nc.sync.dma_start(out=Cb[:, 0:hw],
                                  in_=x[b, a * 85 + 5: a * 85 + 85, :])
        # field loads, one DMA per channel (DMA APs are limited to 3 dims):
        # channel ch of (b, a) -> T1 row ch*32 + a*4 + b (ch 0..2),
        #                        T2 row (ch-3)*12 + a*4 + b (ch 3..4)
        l1 = {}
        for ch in range(3):
            for a in range(3):
                d = nc.sync.dma_start(
                    out=_ap(T1, (ch * 32 + 4 * a) * HWMAX,
                            [[HWMAX, 4], [1, hw]]),
                    in_=_ap(x, (a * 85 + ch) * hw, [[255 * hw, 4], [1, hw]]))
                l1[(ch, a)] = d
        l2 = {}
        for ch in range(2):
            for a in range(3):
                d = nc.sync.dma_start(
                    out=_ap(T2, (ch * 12 + 4 * a) * HWMAX,
                            [[HWMAX, 4], [1, hw]]),
                    in_=_ap(x, (a * 85 + 3 + ch) * hw,
                            [[255 * hw, 4], [1, hw]]))
                l2[(ch, a)] = d

        # --- field math (full-width, batched over the 12 (b,a)) -----------
        i_gt = nc.vector.tensor_scalar(M[:, 0:hw], T1[0:12, 0:hw],
                                       thresh_rep, None,
                                       op0=mybir.AluOpType.is_gt)
        _dep(i_gt, *[l1[(0, a)] for a in range(3)])
        i_sig = nc.scalar.activation(T1[0:12, 0:hw], T1[0:12, 0:hw],
                                     mybir.ActivationFunctionType.Sigmoid)
        _dep(i_sig, i_gt, *[l1[(0, a)] for a in range(3)])
        i_xadd = nc.vector.tensor_add(T1[32:44, 0:hw], T1[32:44, 0:hw],
                                      Gp[32:44, 0:hw])
        _dep(i_xadd, d_gp32, *[l1[(1, a)] for a in range(3)])
        i_xmul = nc.vector.tensor_scalar_mul(T1[32:44, 0:hw],
                                             T1[32:44, 0:hw],
                                             scale_rep[s][32:44])
        _dep(i_xmul, scale_rep[(s, "w32")])
        i_yadd = nc.vector.tensor_add(T1[64:76, 0:hw], T1[64:76, 0:hw],
                                      Gp[64:76, 0:hw])
        _dep(i_yadd, d_gp64, *[l1[(2, a)] for a in range(3)])
        i_ymul = nc.vector.tensor_scalar_mul(T1[64:76, 0:hw],
                                             T1[64:76, 0:hw],
                                             scale_rep[s][64:76])
        _dep(i_ymul, scale_rep[(s, "w64")])
        i_exp = nc.scalar.activation(T2[:, 0:hw], T2[:, 0:hw],
                                     mybir.ActivationFunctionType.Exp,
                                     bias=lnwh[s])
        _dep(i_exp, lnwh[(s, "w")], *l2.values())

        # --- permute fields -> Gt[b] rows a*6 + f --------------------------
        perms = {}
        for b in range(B_LOCAL):
            plist = []
            for a in range(3):
                # T1 (obj,x,y): rows 32f + 4a + b -> rows a*6 + f (f=0,1,2)
                p1 = nc.sync.dma_start(
                    out=_ap(Gt[b], a * 6 * CPADW_MAX,
                            [[CPADW_MAX, 3], [1, hw]]),
                    in_=_ap(T1, (4 * a + b) * HWMAX,
                            [[32 * HWMAX, 3], [1, hw]]))
                _dep(p1, i_sig, i_xmul, i_ymul)
                # T2 (w,h): rows 12f + 4a + b -> rows a*6 + 3 + f
                p2 = nc.sync.dma_start(
                    out=_ap(Gt[b], (a * 6 + 3) * CPADW_MAX,
                            [[CPADW_MAX, 2], [1, hw]]),
                    in_=_ap(T2, (4 * a + b) * HWMAX,
                            [[12 * HWMAX, 2], [1, hw]]))
                _dep(p2, i_exp)
                plist.extend([p1, p2])
            # M: rows a*4 + b -> rows 18 + a
            p3 = nc.sync.dma_start(
                out=_ap(Gt[b], 18 * CPADW_MAX, [[CPADW_MAX, 3], [1, hw]]),
                in_=_ap(M, b * HWMAX, [[4 * HWMAX, 3], [1, hw]]))
            _dep(p3, i_gt)
            plist.append(p3)
            perms[b] = tuple(plist)

        # --- per-group assembly: sc52 -> one b per PSUM bank, else all 4 --
        groups = ([[b] for b in range(B_LOCAL)] if s == 52
                  else [list(range(B_LOCAL))])
        for grp in groups:
            ob_cols = len(grp) * nch * 21
            Po = po.tile([128, ob_cols], F32, tag="po", name="Po")
            Ob = opool.tile([128, ob_cols], F32, tag="O", name="Ob")
            idx_of = {}
            for k, b in enumerate(grp):
                bcol0 = k * nch * 21
                IDX = ipool.tile([128, nch, 3, 8], U32, tag="IDX", name="IDX")
                idx_of[b] = IDX

                # argmax over classes, chunks grouped 6 per PSUM bank
                for a in range(3):
                    Cb = C[(b, a)]
                    for g0 in range(0, nch, 6):
                        gn = min(6, nch - g0)
                        Pb = pc.tile([128, 480], F32, tag="pc", name="Pb")
                        for jj in range(gn):
                            c = g0 + jj
                            tr = nc.tensor.transpose(
                                Pb[:, jj * 80:(jj + 1) * 80],
                                Cb[:, c * 128:(c + 1) * 128],
                                id_sb[0:80, 0:80])
                            if c == nch - 1 and (b, a) in c_ms:
                                _dep(tr, c_ms[(b, a)])
                        m = mpool.tile([128, 6], F32, tag="m", name="m")
                        Pr = Pb.rearrange("p (g k) -> p g k", k=80)
                        nc.vector.tensor_reduce(
                            m[:, 0:gn], Pr[:, 0:gn, :],
                            axis=mybir.AxisListType.X,
                            op=mybir.AluOpType.max)
                        S = spool.tile([128, 480], F32, tag="S", name="S")
                        nc.scalar.activation(
                            S[:, 0:gn * 80], Pb[:, 0:gn * 80],
                            mybir.ActivationFunctionType.Copy)
                        for jj in range(gn):
                            c = g0 + jj
                            nc.vector.max_index(
                                IDX[:, c, a, :],
                                m[:, jj:jj + 1].to_broadcast([128, 8]),
                                S[:, jj * 80:(jj + 1) * 80])

                # output transposes for this b
                for c in range(nch):
                    tr = nc.tensor.transpose(
                        Po[:, bcol0 + c * 21: bcol0 + (c + 1) * 21],
                        Gt[b][:, c * 128:(c + 1) * 128],
                        id_sb[0:21, 0:21])
                    _dep(tr, gt_ms[b], *perms[b])

            nc.scalar.activation(Ob, Po, mybir.ActivationFunctionType.Copy)
            Obr = Ob.rearrange("p (k c r) -> p k c r", c=nch, r=21)
            for k, b in enumerate(grp):
                # cls (uint32 -> f32) into cols k*nch*21 + c*21 + a*6 + 5
                for a in range(3):
                    nc.vector.tensor_copy(
                        Obr[:, k, :, 5 + 6 * a],
                        idx_of[b][:, :, a, 0])
            for k, b in enumerate(grp):
                _dma_out(nc, boxes_o, mask_o, Obr, i, hw, nch, b, k)


def _dma_out(nc, boxes_o, mask_o, Obr, i, hw, nch, b, k):
    base = LOCAL_BASES[i]
    nf = hw // 128                # full 128-candidate chunks
    cw = hw - nf * 128            # ragged tail width
    bo = boxes_o[:, :]
    mo = mask_o[:]
    row0 = base + b * hw * 3
    if nf > 0:
        nc.sync.dma_start(
            out=_ap(bo, row0 * 6, [[18, 128], [2304, nf], [1, 18]]),
            in_=Obr[:, k, 0:nf, 0:18])
        nc.sync.dma_start(
            out=_ap(mo, row0, [[3, 128], [384, nf], [1, 3]]),
            in_=Obr[:, k, 0:nf, 18:21])
    if cw > 0:
        nc.sync.dma_start(
            out=_ap(bo, (row0 + nf * 384) * 6, [[18, cw], [1, 18]]),
            in_=Obr[0:cw, k, nf, 0:18])
        nc.sync.dma_start(
            out=_ap(mo, row0 + nf * 384, [[3, cw], [1, 3]]),
            in_=Obr[0:cw, k, nf, 18:21])


_PROGRAM = None
_LOCK = threading.Lock()


def _get_program():
    global _PROGRAM
    with _LOCK:
        if _PROGRAM is None:
            _PROGRAM = _build_program()
    return _PROGRAM


def _host_consts(anchors_13, anchors_26, anchors_52, thresh, case):
    case_f = float(np.asarray(case).reshape(-1)[0])
    anchors = {13: np.asarray(anchors_13, np.float32),
               26: np.asarray(anchors_26, np.float32),
               52: np.asarray(anchors_52, np.float32)}
    nblob = 156 + 24 * sum(HWS)
    blob = np.zeros(nblob, np.float32)
    blob[0:12] = np.float32(np.asarray(thresh).reshape(-1)[0])
    goff = 156
    for i, (s, t) in enumerate(SCALES):
        blob[12 + i * 24: 12 + i * 24 + 24] = np.float32(t / case_f)
        a = np.maximum(anchors[s].astype(np.float64) / case_f, 1e-38)
        la = np.log(a).astype(np.float32)
        # rows 0..11: ln(aw) at partition ba = a*4 + b; rows 12..23: ln(ah)
        lw = np.repeat(la[:, 0], 4)
        lh = np.repeat(la[:, 1], 4)
        blob[84 + i * 24: 84 + i * 24 + 12] = lw
        blob[84 + i * 24 + 12: 84 + i * 24 + 24] = lh
        hwn = s * s
        idx = np.arange(hwn, dtype=np.float32)
        gx = np.tile(idx % s, (12, 1))
        gy = np.tile(np.floor(idx / s).astype(np.float32), (12, 1))
        blob[goff: goff + 24 * hwn] = np.concatenate(
            [gx, gy], axis=0).reshape(-1)
        goff += 24 * hwn
    ident = np.eye(128, dtype=np.float32)
    return blob, ident


def make_in_maps(output_13, output_26, output_52, anchors_13, anchors_26,
                 anchors_52, thresh, case):
    blob, ident = _host_consts(anchors_13, anchors_26, anchors_52,
                               thresh, case)
    outs = {13: np.asarray(output_13, np.float32),
            26: np.asarray(output_26, np.float32),
            52: np.asarray(output_52, np.float32)}
    in_maps = []
    for c in range(N_CORES):
        m = {"cblob": blob, "ident": ident}
        for s, _t in SCALES:
            hwn = s * s
            m[f"x{s}"] = np.ascontiguousarray(
                outs[s][c * B_LOCAL:(c + 1) * B_LOCAL].reshape(
                    B_LOCAL, 255, hwn))
        in_maps.append(m)
    return in_maps


def assemble(per_core_results):
    boxes = np.empty((N_FULL, 6), np.float32)
    mask = np.empty(N_FULL, np.float32)
    gbase = 0
    for i, hwn in enumerate(HWS):
        rows_per_b = hwn * 3
        n = B_LOCAL * rows_per_b
        lo = LOCAL_BASES[i]
        for c in range(N_CORES):
            gl = gbase + c * n
            boxes[gl:gl + n] = per_core_results[c]["boxes"][lo:lo + n]
            mask[gl:gl + n] = per_core_results[c]["maskf"][lo:lo + n]
        gbase += N_CORES * n
    return boxes, mask > 0.5


def kernel(output_13, output_26, output_52, anchors_13, anchors_26,
           anchors_52, thresh, case):
    nc = _get_program()
    in_maps = make_in_maps(output_13, output_26, output_52, anchors_13,
                           anchors_26, anchors_52, thresh, case)
    res = run_bass_kernel_spmd(nc, in_maps, core_ids=list(range(N_CORES)))
    return assemble(res.results)


# revision 30
# speedup vs baseline: 1.0383x; 1.0383x over previous
"""YOLOv3-style detection decode kernel for Trainium2 (8 NeuronCores).

kernel(**inputs) takes the FULL unsharded inputs (as produced by
setup_inputs) and returns (boxes [N,6] f32, mask [N] bool) matching the
reference. The batch (32) is sharded 8 ways (4 per core); one SPMD
Bass/Tile program runs on all 8 cores with per-core input maps and the
host reassembles the full outputs.

Per-core pipeline (per scale):
  - contiguous channel-major loads: class slabs [80, HW] per (b, anchor);
    the five box-field channels into a single field tile FT [128, HWMAX]
    at aligned row bases (obj@0, x@32, y@44, w@64, h@76; +4a+b), with
    host-precomputed gx*scale / gy*scale rows at 120/121
  - batched field math: sigmoid(obj), exp(w/h + ln(anchor/case)),
    mask = obj > thresh (into rows 96..107)
  - output assembly as PE matmuls with per-(scale,b) constant
    permutation/affine matrices: out[cand, a*6+f] = sum_k FT[k,cand]*P[k,n]
    (x/y scaling and the grid add are baked into P; cls merged after)
  - exact argmax over the 80 classes: PE data-as-weights transposes
    [80,128] -> [128,80] into PSUM, segmented DVE reduce_max, ACT copy
    PSUM->SBUF, DVE max_index (first-occurrence, matching jnp.argmax)
  - contiguous DMA of boxes (18 f32 per candidate-chunk row) and mask
"""

import threading
from contextlib import ExitStack

import numpy as np

import concourse.bacc as bacc
import concourse.bass as bass
import concourse.mybir as mybir
import concourse.tile as tile
from concourse.bass_utils import run_bass_kernel_spmd
from concourse.tile import add_dep_helper


def _dep(frm, *tos):
    """Explicit dependency edges: raw (strided-partition) APs are tracked
    at last-writer granularity only, so readers depend on every writer
    explicitly."""
    f = getattr(frm, "ins", frm)
    for t in tos:
        add_dep_helper(f, getattr(t, "ins", t), reason="raw-ap-dep")


F32 = mybir.dt.float32
U32 = mybir.dt.uint32

SCALES = [(13, 32.0), (26, 16.0), (52, 8.0)]
B_LOCAL = 4
N_CORES = 8

HWS = [s * s for s, _ in SCALES]              # 169, 676, 2704
LOCAL_BASES = []
_acc = 0
for _hw in HWS:
    LOCAL_BASES.append(_acc)
    _acc += B_LOCAL * _hw * 3
N_LOCAL = _acc                                 # 42588
N_FULL = N_LOCAL * N_CORES                     # 340704

HWMAX = max(HWS)
CPADW_MAX = ((HWMAX + 127) // 128) * 128

# FT row bases: channel ch of (a, b) lands at FROW[ch] + 4a + b
FROW = [0, 32, 44, 64, 76]   # obj, x, y, w, h
MROW = 96                    # mask rows
GROW = 120                   # gx*scale row; gy*scale at 121

# consts blob layout (element offsets)
OFF_LNWH = 12                         # per scale: 24 (ln aw x12, ln ah x12)
OFF_PB = 84                           # 12 x [128,18] (scale-major, b-minor)
OFF_PM = OFF_PB + 12 * 128 * 18       # 4 x [128,3]
OFF_GXY = OFF_PM + 4 * 128 * 3        # per scale: [2, hw]
NBLOB = OFF_GXY + 2 * sum(HWS)

PAD_VAL = -1.0e30


def _ap(t, offset, ap):
    return bass.AP(tensor=t.tensor if isinstance(t, bass.AP) else t,
                   offset=offset, ap=ap)


def _build_program():
    nc = bacc.Bacc()
    xs = {}
    for (s, _t), hw in zip(SCALES, HWS):
        xs[s] = nc.dram_tensor(f"x{s}", [B_LOCAL, 255, hw], F32,
                               kind="ExternalInput")
    cblob = nc.dram_tensor("cblob", [NBLOB], F32, kind="ExternalInput")
    ident = nc.dram_tensor("ident", [128, 128], F32, kind="ExternalInput")
    boxes_o = nc.dram_tensor("boxes", [N_LOCAL, 6], F32, kind="ExternalOutput")
    mask_o = nc.dram_tensor("maskf", [N_LOCAL], F32, kind="ExternalOutput")

    import os
    prev = os.environ.get("BY_DEFAULT_DISABLE_SUBTILE_DEPS")
    os.environ["BY_DEFAULT_DISABLE_SUBTILE_DEPS"] = "1"
    try:
        with tile.TileContext(nc) as tc:
            with ExitStack() as ctx:
                _emit(ctx, tc, nc, xs, cblob, ident, boxes_o, mask_o)
    finally:
        if prev is None:
            os.environ.pop("BY_DEFAULT_DISABLE_SUBTILE_DEPS", None)
        else:
            os.environ["BY_DEFAULT_DISABLE_SUBTILE_DEPS"] = prev
    nc.compile()
    return nc


def _emit(ctx, tc, nc, xs, cblob, ident, boxes_o, mask_o):
    consts = ctx.enter_context(tc.tile_pool(name="consts", bufs=1))
    fpool = ctx.enter_context(tc.tile_pool(name="fpool", bufs=1))
    cpool = ctx.enter_context(tc.tile_pool(name="cpool", bufs=6))
    spool = ctx.enter_context(tc.tile_pool(name="spool", bufs=3))
    mpool = ctx.enter_context(tc.tile_pool(name="mpool", bufs=3))
    ipool = ctx.enter_context(tc.tile_pool(name="ipool", bufs=5))
    opool = ctx.enter_context(tc.tile_pool(name="opool", bufs=2))
    pc = ctx.enter_context(tc.tile_pool(name="pc", bufs=2, space="PSUM"))
    pob = ctx.enter_context(tc.tile_pool(name="pob", bufs=2, space="PSUM"))
    pom = ctx.enter_context(tc.tile_pool(name="pom", bufs=2, space="PSUM"))

    pr = cblob[:]
    id_sb = consts.tile([128, 128], F32, tag="id", name="id_sb")
    nc.sync.dma_start(out=id_sb, in_=ident[:, :])
    thresh_rep = consts.tile([12, 1], F32, tag="thresh", name="thresh_rep")
    nc.sync.dma_start(out=thresh_rep, in_=pr[0:12])

    # exp bias tile: rows 64..87 <- [ln(aw/case) x12, ln(ah/case) x12]
    lnwh, d_lnwh = {}, {}
    for i, (s, _t) in enumerate(SCALES):
        lb = consts.tile([88, 1], F32, tag=f"ln{s}", name=f"ln{s}")
        d = nc.sync.dma_start(
            out=_ap(lb, 64, [[1, 24], [1, 1]]),
            in_=pr[OFF_LNWH + i * 24: OFF_LNWH + i * 24 + 24])
        lnwh[s], d_lnwh[s] = lb, d

    # permutation/affine matrices
    PB = {}
    for i, (s, _t) in enumerate(SCALES):
        for b in range(B_LOCAL):
            t = consts.tile([128, 18], F32, tag=f"PB{s}{b}",
                            name=f"PB{s}{b}")
            o = OFF_PB + (i * B_LOCAL + b) * 128 * 18
            nc.sync.dma_start(out=t, in_=pr[o: o + 128 * 18])
            PB[(s, b)] = t
    PM = {}
    for b in range(B_LOCAL):
        t = consts.tile([128, 3], F32, tag=f"PM{b}", name=f"PM{b}")
        o = OFF_PM + b * 128 * 3
        nc.sync.dma_start(out=t, in_=pr[o: o + 128 * 3])
        PM[b] = t

    # static field tile (memset once: unused rows must be exact 0.0 for the
    # assembly matmuls; used regions are overwritten every scale)
    FT = fpool.tile([128, CPADW_MAX], F32, tag="FT", name="FT")
    ft_ms = nc.gpsimd.memset(FT, 0.0)

    goff = OFF_GXY
    for i, ((s, t), hw) in enumerate(zip(SCALES, HWS)):
        if i > 0:
            tc.strict_bb_all_engine_barrier()
        nch = (hw + 127) // 128
        cpadw = nch * 128
        x = xs[s][:, :, :]

        # --- loads ---------------------------------------------------------
        d_gxy = nc.sync.dma_start(
            out=_ap(FT, GROW * CPADW_MAX, [[CPADW_MAX, 2], [1, hw]]),
            in_=pr[goff: goff + 2 * hw])
        if i == 0:
            _dep(d_gxy, ft_ms)
        goff += 2 * hw

        fl = {}
        for ch in range(5):
            for a in range(3):
                d = nc.sync.dma_start(
                    out=_ap(FT, (FROW[ch] + 4 * a) * CPADW_MAX,
                            [[CPADW_MAX, 4], [1, hw]]),
                    in_=_ap(x, (a * 85 + ch) * hw, [[255 * hw, 4], [1, hw]]))
                if i == 0:
                    _dep(d, ft_ms)
                fl[(ch, a)] = d

        C = {}
        c_ms = {}
        for b in range(B_LOCAL):
            for a in range(3):
                Cb = cpool.tile([80, cpadw], F32, tag="C", name="Cb")
                C[(b, a)] = Cb
                if cpadw > hw:
                    c_ms[(b, a)] = nc.gpsimd.memset(Cb[:, hw:], PAD_VAL)
                nc.sync.dma_start(out=Cb[:, 0:hw],
                                  in_=x[b, a * 85 + 5: a * 85 + 85, :])

        # --- field math ----------------------------------------------------
        i_gt = nc.vector.tensor_scalar(FT[MROW:MROW + 12, 0:hw],
                                       FT[0:12, 0:hw], thresh_rep, None,
                                       op0=mybir.AluOpType.is_gt)
        _dep(i_gt, *[fl[(0, a)] for a in range(3)])
        i_sig = nc.scalar.activation(FT[0:12, 0:hw], FT[0:12, 0:hw],
                                     mybir.ActivationFunctionType.Sigmoid)
        _dep(i_sig, i_gt, *[fl[(0, a)] for a in range(3)])
        i_exp = nc.scalar.activation(FT[64:88, 0:hw], FT[64:88, 0:hw],
                                     mybir.ActivationFunctionType.Exp,
                                     bias=lnwh[s][64:88])
        _dep(i_exp, d_lnwh[s],
             *[fl[(ch, a)] for ch in (3, 4) for a in range(3)])
        mm_deps = ([i_sig, i_exp, d_gxy]
                   + [fl[(ch, a)] for ch in (1, 2) for a in range(3)])

        # --- per-group assembly + argmax ----------------------------------
        groups = ([[b] for b in range(B_LOCAL)] if s == 52
                  else [list(range(B_LOCAL))])
        for grp in groups:
            ng = len(grp)
            Pbx = pob.tile([128, ng * nch * 18], F32, tag="pob", name="Pbx")
            Pms = pom.tile([128, ng * nch * 3], F32, tag="pom", name="Pms")
            Obox = opool.tile([128, ng * nch * 18], F32, tag="Ob",
                              name="Obox")
            Omask = opool.tile([128, ng * nch * 3], F32, tag="Om",
                               name="Omask")
            idx_of = {}
            for k, b in enumerate(grp):
                IDX = ipool.tile([128, nch, 3, 8], U32, tag="IDX",
                                 name="IDX")
                idx_of[b] = IDX

                # class argmax: transposes packed 12 chunks / 2-bank tile
                for a in range(3):
                    Cb = C[(b, a)]
                    for g0 in range(0, nch, 12):
                        gn = min(12, nch - g0)
                        nb = (gn + 5) // 6
                        Pb = pc.tile([128, 1024], F32, tag="pc", name="Pb")
                        for jj in range(gn):
                            c = g0 + jj
                            col = (jj // 6) * 512 + (jj % 6) * 80
                            tr = nc.tensor.transpose(
                                Pb[:, col: col + 80],
                                Cb[:, c * 128:(c + 1) * 128],
                                id_sb[0:80, 0:80])
                            if c == nch - 1 and (b, a) in c_ms:
                                _dep(tr, c_ms[(b, a)])
                        m = mpool.tile([128, 12], F32, tag="m", name="m")
                        nbf, rem = gn // 6, gn % 6
                        if nbf:
                            Pr = Pb.rearrange(
                                "p (bk r) -> p bk r", bk=2)[
                                :, 0:nbf, 0:480].rearrange(
                                "p bk (g c) -> p bk g c", c=80)
                            nc.vector.tensor_reduce(
                                m[:, 0: nbf * 6], Pr,
                                axis=mybir.AxisListType.X,
                                op=mybir.AluOpType.max)
                        if rem:
                            Pr2 = Pb[:, nbf * 512: nbf * 512 + rem * 80
                                     ].rearrange("p (g c) -> p g c", c=80)
                            nc.vector.tensor_reduce(
                                m[:, nbf * 6: nbf * 6 + rem], Pr2,
                                axis=mybir.AxisListType.X,
                                op=mybir.AluOpType.max)
                        S = spool.tile([128, 1024], F32, tag="S", name="S")
                        for bk in range(nbf):
                            nc.scalar.activation(
                                S[:, bk * 512: bk * 512 + 480],
                                Pb[:, bk * 512: bk * 512 + 480],
                                mybir.ActivationFunctionType.Copy)
                        if rem:
                            nc.scalar.activation(
                                S[:, nbf * 512: nbf * 512 + rem * 80],
                                Pb[:, nbf * 512: nbf * 512 + rem * 80],
                                mybir.ActivationFunctionType.Copy)
                        for jj in range(gn):
                            c = g0 + jj
                            col = (jj // 6) * 512 + (jj % 6) * 80
                            nc.vector.max_index(
                                IDX[:, c, a, :],
                                m[:, jj:jj + 1].to_broadcast([128, 8]),
                                S[:, col: col + 80])

                # output assembly matmuls
                for c in range(nch):
                    mm1 = nc.tensor.matmul(
                        Pbx[:, (k * nch + c) * 18: (k * nch + c + 1) * 18],
                        FT[:, c * 128:(c + 1) * 128],
                        PB[(s, b)][:, :], start=True, stop=True)
                    _dep(mm1, *mm_deps)
                    mm2 = nc.tensor.matmul(
                        Pms[:, (k * nch + c) * 3: (k * nch + c + 1) * 3],
                        FT[:, c * 128:(c + 1) * 128],
                        PM[b][:, :], start=True, stop=True)
                    _dep(mm2, i_gt)

            nc.scalar.activation(Obox, Pbx,
                                 mybir.ActivationFunctionType.Copy)
            nc.scalar.activation(Omask, Pms,
                                 mybir.ActivationFunctionType.Copy)
            Obr = Obox.rearrange("p (c r) -> p c r", r=18)
            for k, b in enumerate(grp):
                for a in range(3):
                    nc.vector.tensor_copy(
                        Obr[:, k * nch:(k + 1) * nch, 5 + 6 * a],
                        idx_of[b][:, :, a, 0])
            for k, b in enumerate(grp):
                _dma_out(nc, boxes_o, mask_o, Obox, Omask, i, hw, nch, b, k)


def _dma_out(nc, boxes_o, mask_o, Obox, Omask, i, hw, nch, b, k):
    base = LOCAL_BASES[i]
    nf = hw // 128
    cw = hw - nf * 128
    bo = boxes_o[:, :]
    mo = mask_o[:]
    row0 = base + b * hw * 3
    ob = Obox[:, k * nch * 18:]
    om = Omask[:, k * nch * 3:]
    if nf > 0:
        nc.sync.dma_start(
            out=_ap(bo, row0 * 6, [[18, 128], [2304, nf], [1, 18]]),
            in_=ob[:, 0: nf * 18])
        nc.sync.dma_start(
            out=_ap(mo, row0, [[3, 128], [384, nf], [1, 3]]),
            in_=om[:, 0: nf * 3])
    if cw > 0:
        nc.sync.dma_start(
            out=_ap(bo, (row0 + nf * 384) * 6, [[18, cw], [1, 18]]),
            in_=ob[0:cw, nf * 18: (nf + 1) * 18])
        nc.sync.dma_start(
            out=_ap(mo, row0 + nf * 384, [[3, cw], [1, 3]]),
            in_=om[0:cw, nf * 3: (nf + 1) * 3])


_PROGRAM = None
_LOCK = threading.Lock()


def _get_program():
    global _PROGRAM
    with _LOCK:
        if _PROGRAM is None:
            _PROGRAM = _build_program()
    return _PROGRAM


def _host_consts(anchors_13, anchors_26, anchors_52, thresh, case):
    case_f = float(np.asarray(case).reshape(-1)[0])
    anchors = {13: np.asarray(anchors_13, np.float32),
               26: np.asarray(anchors_26, np.float32),
               52: np.asarray(anchors_52, np.float32)}
    blob = np.zeros(NBLOB, np.float32)
    blob[0:12] = np.float32(np.asarray(thresh).reshape(-1)[0])
    goff = OFF_GXY
    for i, (s, t) in enumerate(SCALES):
        scale = np.float32(t / case_f)
        a = np.maximum(anchors[s].astype(np.float64) / case_f, 1e-38)
        la = np.log(a).astype(np.float32)
        blob[OFF_LNWH + i * 24: OFF_LNWH + i * 24 + 12] = np.repeat(la[:, 0], 4)
        blob[OFF_LNWH + i * 24 + 12: OFF_LNWH + i * 24 + 24] = (
            np.repeat(la[:, 1], 4))
        hwn = s * s
        idx = np.arange(hwn, dtype=np.float32)
        blob[goff: goff + hwn] = (idx % s).astype(np.float32) * scale
        blob[goff + hwn: goff + 2 * hwn] = (
            np.floor(idx / s).astype(np.float32) * scale)
        goff += 2 * hwn
        for b in range(B_LOCAL):
            P = np.zeros((128, 18), np.float32)
            for an in range(3):
                P[FROW[0] + 4 * an + b, an * 6 + 0] = 1.0
                P[FROW[1] + 4 * an + b, an * 6 + 1] = scale
                P[FROW[2] + 4 * an + b, an * 6 + 2] = scale
                P[FROW[3] + 4 * an + b, an * 6 + 3] = 1.0
                P[FROW[4] + 4 * an + b, an * 6 + 4] = 1.0
                P[GROW, an * 6 + 1] = 1.0
                P[GROW + 1, an * 6 + 2] = 1.0
            o = OFF_PB + (i * B_LOCAL + b) * 128 * 18
            blob[o: o + 128 * 18] = P.reshape(-1)
    for b in range(B_LOCAL):
        P = np.zeros((128, 3), np.float32)
        for an in range(3):
            P[MROW + 4 * an + b, an] = 1.0
        o = OFF_PM + b * 128 * 3
        blob[o: o + 128 * 3] = P.reshape(-1)
    ident = np.eye(128, dtype=np.float32)
    return blob, ident


def make_in_maps(output_13, output_26, output_52, anchors_13, anchors_26,
                 anchors_52, thresh, case):
    blob, ident = _host_consts(anchors_13, anchors_26, anchors_52,
                               thresh, case)
    outs = {13: np.asarray(output_13, np.float32),
            26: np.asarray(output_26, np.float32),
            52: np.asarray(output_52, np.float32)}
    in_maps = []
    for c in range(N_CORES):
        m = {"cblob": blob, "ident": ident}
        for s, _t in SCALES:
            hwn = s * s
            m[f"x{s}"] = np.ascontiguousarray(
                outs[s][c * B_LOCAL:(c + 1) * B_LOCAL].reshape(
                    B_LOCAL, 255, hwn))
        in_maps.append(m)
    return in_maps


def assemble(per_core_results):
    boxes = np.empty((N_FULL, 6), np.float32)
    mask = np.empty(N_FULL, np.float32)
    gbase = 0
    for i, hwn in enumerate(HWS):
        rows_per_b = hwn * 3
        n = B_LOCAL * rows_per_b
        lo = LOCAL_BASES[i]
        for c in range(N_CORES):
            gl = gbase + c * n
            boxes[gl:gl + n] = per_core_results[c]["boxes"][lo:lo + n]
            mask[gl:gl + n] = per_core_results[c]["maskf"][lo:lo + n]
        gbase += N_CORES * n
    return boxes, mask > 0.5


def kernel(output_13, output_26, output_52, anchors_13, anchors_26,
           anchors_52, thresh, case):
    nc = _get_program()
    in_maps = make_in_maps(output_13, output_26, output_52, anchors_13,
                           anchors_26, anchors_52, thresh, case)
    res = run_bass_kernel_spmd(nc, in_maps, core_ids=list(range(N_CORES)))
    return assemble(res.results)


# revision 31
# speedup vs baseline: 12311.7791x; 11857.8610x over previous
"""YOLOv3-style detection decode kernel for Trainium2 (8 NeuronCores).

kernel(**inputs) takes the FULL unsharded inputs (as produced by
setup_inputs) and returns (boxes [N,6] f32, mask [N] bool) matching the
reference. The batch (32) is sharded 8 ways (4 per core); one SPMD
Bass/Tile program runs on all 8 cores with per-core input maps and the
host reassembles the full outputs.

Per-core pipeline (per scale):
  - contiguous channel-major loads: class slabs [80, HW] per (b, anchor);
    the five box-field channels into a single field tile FT [128, HWMAX]
    at aligned row bases (obj@0, x@32, y@44, w@64, h@76; +4a+b), with
    host-precomputed gx*scale / gy*scale rows at 120/121
  - batched field math: sigmoid(obj), exp(w/h + ln(anchor/case)),
    mask = obj > thresh (into rows 96..107)
  - output assembly as PE matmuls with per-(scale,b) constant
    permutation/affine matrices: out[cand, a*6+f] = sum_k FT[k,cand]*P[k,n]
    (x/y scaling and the grid add are baked into P; cls merged after)
  - exact argmax over the 80 classes: PE data-as-weights transposes
    [80,128] -> [128,80] into PSUM, segmented DVE reduce_max, ACT copy
    PSUM->SBUF, DVE max_index (first-occurrence, matching jnp.argmax)
  - contiguous DMA of boxes (18 f32 per candidate-chunk row) and mask
"""

import threading
from contextlib import ExitStack

import numpy as np

import concourse.bacc as bacc
import concourse.bass as bass
import concourse.mybir as mybir
import concourse.tile as tile
from concourse.bass_utils import run_bass_kernel_spmd
from concourse.tile import add_dep_helper


def _dep(frm, *tos):
    """Explicit dependency edges: raw (strided-partition) APs are tracked
    at last-writer granularity only, so readers depend on every writer
    explicitly."""
    f = getattr(frm, "ins", frm)
    for t in tos:
        add_dep_helper(f, getattr(t, "ins", t), reason="raw-ap-dep")


F32 = mybir.dt.float32
U32 = mybir.dt.uint32

SCALES = [(13, 32.0), (26, 16.0), (52, 8.0)]
B_LOCAL = 4
N_CORES = 8

HWS = [s * s for s, _ in SCALES]              # 169, 676, 2704
LOCAL_BASES = []
_acc = 0
for _hw in HWS:
    LOCAL_BASES.append(_acc)
    _acc += B_LOCAL * _hw * 3
N_LOCAL = _acc                                 # 42588
N_FULL = N_LOCAL * N_CORES                     # 340704

HWMAX = max(HWS)
CPADW_MAX = ((HWMAX + 127) // 128) * 128

# FT row bases: channel ch of (a, b) lands at FROW[ch] + 4a + b
FROW = [0, 32, 44, 64, 76]   # obj, x, y, w, h
MROW = 96                    # mask rows
GROW = 120                   # gx*scale row; gy*scale at 121

# consts blob layout (element offsets)
OFF_LNWH = 12                         # per scale: 24 (ln aw x12, ln ah x12)
OFF_PB = 84                           # 12 x [128,18] (scale-major, b-minor)
OFF_PM = OFF_PB + 12 * 128 * 18       # 4 x [128,3]
OFF_GXY = OFF_PM + 4 * 128 * 3        # per scale: [2, hw]
OFF_SCL = OFF_GXY + 2 * sum(HWS)      # per scale: scale x128
NBLOB = OFF_SCL + 3 * 128

PAD_VAL = -1.0e30


def _ap(t, offset, ap):
    return bass.AP(tensor=t.tensor if isinstance(t, bass.AP) else t,
                   offset=offset, ap=ap)


def _build_program():
    nc = bacc.Bacc()
    xs = {}
    for (s, _t), hw in zip(SCALES, HWS):
        xs[s] = nc.dram_tensor(f"x{s}", [B_LOCAL, 255, hw], F32,
                               kind="ExternalInput")
    cblob = nc.dram_tensor("cblob", [NBLOB], F32, kind="ExternalInput")
    ident = nc.dram_tensor("ident", [128, 128], F32, kind="ExternalInput")
    boxes_o = nc.dram_tensor("boxes", [N_LOCAL, 6], F32, kind="ExternalOutput")
    mask_o = nc.dram_tensor("maskf", [N_LOCAL], F32, kind="ExternalOutput")

    import os
    prev = os.environ.get("BY_DEFAULT_DISABLE_SUBTILE_DEPS")
    os.environ["BY_DEFAULT_DISABLE_SUBTILE_DEPS"] = "1"
    try:
        with tile.TileContext(nc) as tc:
            with ExitStack() as ctx:
                _emit(ctx, tc, nc, xs, cblob, ident, boxes_o, mask_o)
    finally:
        if prev is None:
            os.environ.pop("BY_DEFAULT_DISABLE_SUBTILE_DEPS", None)
        else:
            os.environ["BY_DEFAULT_DISABLE_SUBTILE_DEPS"] = prev
    nc.compile()
    return nc


def _emit(ctx, tc, nc, xs, cblob, ident, boxes_o, mask_o):
    consts = ctx.enter_context(tc.tile_pool(name="consts", bufs=1))
    fpool = ctx.enter_context(tc.tile_pool(name="fpool", bufs=1))
    cpool = ctx.enter_context(tc.tile_pool(name="cpool", bufs=6))
    spool = ctx.enter_context(tc.tile_pool(name="spool", bufs=3))
    mpool = ctx.enter_context(tc.tile_pool(name="mpool", bufs=3))
    ipool = ctx.enter_context(tc.tile_pool(name="ipool", bufs=5))
    opool = ctx.enter_context(tc.tile_pool(name="opool", bufs=2))
    pc = ctx.enter_context(tc.tile_pool(name="pc", bufs=2, space="PSUM"))
    pob = ctx.enter_context(tc.tile_pool(name="pob", bufs=2, space="PSUM"))
    pom = ctx.enter_context(tc.tile_pool(name="pom", bufs=2, space="PSUM"))

    pr = cblob[:]
    id_sb = consts.tile([128, 128], F32, tag="id", name="id_sb")
    nc.sync.dma_start(out=id_sb, in_=ident[:, :])
    thresh_rep = consts.tile([12, 1], F32, tag="thresh", name="thresh_rep")
    nc.sync.dma_start(out=thresh_rep, in_=pr[0:12])

    # per-scale scale value replicated on all partitions (cx/cy scaling)
    scl128 = {}
    for i, (s, _t) in enumerate(SCALES):
        t = consts.tile([128, 1], F32, tag=f"sc{s}", name=f"sc{s}")
        nc.sync.dma_start(out=t,
                          in_=pr[OFF_SCL + i * 128: OFF_SCL + (i + 1) * 128])
        scl128[s] = t

    # exp bias tile: rows 64..87 <- [ln(aw/case) x12, ln(ah/case) x12]
    lnwh, d_lnwh = {}, {}
    for i, (s, _t) in enumerate(SCALES):
        lb = consts.tile([88, 1], F32, tag=f"ln{s}", name=f"ln{s}")
        d = nc.sync.dma_start(
            out=_ap(lb, 64, [[1, 24], [1, 1]]),
            in_=pr[OFF_LNWH + i * 24: OFF_LNWH + i * 24 + 24])
        lnwh[s], d_lnwh[s] = lb, d

    # permutation/affine matrices
    PB = {}
    for i, (s, _t) in enumerate(SCALES):
        for b in range(B_LOCAL):
            t = consts.tile([128, 18], F32, tag=f"PB{s}{b}",
                            name=f"PB{s}{b}")
            o = OFF_PB + (i * B_LOCAL + b) * 128 * 18
            nc.sync.dma_start(out=t, in_=pr[o: o + 128 * 18])
            PB[(s, b)] = t
    PM = {}
    for b in range(B_LOCAL):
        t = consts.tile([128, 3], F32, tag=f"PM{b}", name=f"PM{b}")
        o = OFF_PM + b * 128 * 3
        nc.sync.dma_start(out=t, in_=pr[o: o + 128 * 3])
        PM[b] = t

    # static field tile (memset once: unused rows must be exact 0.0 for the
    # assembly matmuls; used regions are overwritten every scale)
    FT = fpool.tile([128, CPADW_MAX], F32, tag="FT", name="FT")
    ft_ms = nc.gpsimd.memset(FT, 0.0)

    goff = OFF_GXY
    for i, ((s, t), hw) in enumerate(zip(SCALES, HWS)):
        if i > 0:
            tc.strict_bb_all_engine_barrier()
        nch = (hw + 127) // 128
        cpadw = nch * 128
        x = xs[s][:, :, :]

        # --- loads ---------------------------------------------------------
        d_gxy = nc.sync.dma_start(
            out=_ap(FT, GROW * CPADW_MAX, [[CPADW_MAX, 2], [1, hw]]),
            in_=pr[goff: goff + 2 * hw])
        if i == 0:
            _dep(d_gxy, ft_ms)
        goff += 2 * hw

        fl = {}
        for ch in range(5):
            for a in range(3):
                d = nc.sync.dma_start(
                    out=_ap(FT, (FROW[ch] + 4 * a) * CPADW_MAX,
                            [[CPADW_MAX, 4], [1, hw]]),
                    in_=_ap(x, (a * 85 + ch) * hw, [[255 * hw, 4], [1, hw]]))
                if i == 0:
                    _dep(d, ft_ms)
                fl[(ch, a)] = d

        C = {}
        c_ms = {}
        for b in range(B_LOCAL):
            for a in range(3):
                Cb = cpool.tile([80, cpadw], F32, tag="C", name="Cb")
                C[(b, a)] = Cb
                if cpadw > hw:
                    c_ms[(b, a)] = nc.gpsimd.memset(Cb[:, hw:], PAD_VAL)
                nc.sync.dma_start(out=Cb[:, 0:hw],
                                  in_=x[b, a * 85 + 5: a * 85 + 85, :])

        # --- field math ----------------------------------------------------
        i_gt = nc.vector.tensor_scalar(FT[MROW:MROW + 12, 0:hw],
                                       FT[0:12, 0:hw], thresh_rep, None,
                                       op0=mybir.AluOpType.is_gt)
        _dep(i_gt, *[fl[(0, a)] for a in range(3)])
        i_sig = nc.scalar.activation(FT[0:12, 0:hw], FT[0:12, 0:hw],
                                     mybir.ActivationFunctionType.Sigmoid)
        _dep(i_sig, i_gt, *[fl[(0, a)] for a in range(3)])
        i_exp = nc.scalar.activation(FT[64:88, 0:hw], FT[64:88, 0:hw],
                                     mybir.ActivationFunctionType.Exp,
                                     bias=lnwh[s][64:88])
        _dep(i_exp, d_lnwh[s],
             *[fl[(ch, a)] for ch in (3, 4) for a in range(3)])
        mm_deps = ([i_sig, i_exp, d_gxy]
                   + [fl[(ch, a)] for ch in (1, 2) for a in range(3)])

        # --- per-group assembly + argmax ----------------------------------
        groups = ([[b] for b in range(B_LOCAL)] if s == 52
                  else [list(range(B_LOCAL))])
        for grp in groups:
            ng = len(grp)
            Pbx = pob.tile([128, ng * nch * 18], F32, tag="pob", name="Pbx")
            Pms = pom.tile([128, ng * nch * 3], F32, tag="pom", name="Pms")
            Obox = opool.tile([128, ng * nch * 18], F32, tag="Ob",
                              name="Obox")
            Omask = opool.tile([128, ng * nch * 3], F32, tag="Om",
                               name="Omask")
            idx_of = {}
            for k, b in enumerate(grp):
                IDX = ipool.tile([128, nch, 3, 8], U32, tag="IDX",
                                 name="IDX")
                idx_of[b] = IDX

                # class argmax: transposes packed 12 chunks / 2-bank tile
                for a in range(3):
                    Cb = C[(b, a)]
                    for g0 in range(0, nch, 12):
                        gn = min(12, nch - g0)
                        nb = (gn + 5) // 6
                        Pb = pc.tile([128, 1024], F32, tag="pc", name="Pb")
                        for jj in range(gn):
                            c = g0 + jj
                            col = (jj // 6) * 512 + (jj % 6) * 80
                            tr = nc.tensor.transpose(
                                Pb[:, col: col + 80],
                                Cb[:, c * 128:(c + 1) * 128],
                                id_sb[0:80, 0:80])
                            if c == nch - 1 and (b, a) in c_ms:
                                _dep(tr, c_ms[(b, a)])
                        m = mpool.tile([128, 12], F32, tag="m", name="m")
                        nbf, rem = gn // 6, gn % 6
                        if nbf:
                            Pr = Pb.rearrange(
                                "p (bk r) -> p bk r", bk=2)[
                                :, 0:nbf, 0:480].rearrange(
                                "p bk (g c) -> p bk g c", c=80)
                            nc.vector.tensor_reduce(
                                m[:, 0: nbf * 6], Pr,
                                axis=mybir.AxisListType.X,
                                op=mybir.AluOpType.max)
                        if rem:
                            Pr2 = Pb[:, nbf * 512: nbf * 512 + rem * 80
                                     ].rearrange("p (g c) -> p g c", c=80)
                            nc.vector.tensor_reduce(
                                m[:, nbf * 6: nbf * 6 + rem], Pr2,
                                axis=mybir.AxisListType.X,
                                op=mybir.AluOpType.max)
                        S = spool.tile([128, 1024], F32, tag="S", name="S")
                        for bk in range(nbf):
                            nc.scalar.activation(
                                S[:, bk * 512: bk * 512 + 480],
                                Pb[:, bk * 512: bk * 512 + 480],
                                mybir.ActivationFunctionType.Copy)
                        if rem:
                            nc.scalar.activation(
                                S[:, nbf * 512: nbf * 512 + rem * 80],
                                Pb[:, nbf * 512: nbf * 512 + rem * 80],
                                mybir.ActivationFunctionType.Copy)
                        for jj in range(gn):
                            c = g0 + jj
                            col = (jj // 6) * 512 + (jj % 6) * 80
                            nc.vector.max_index(
                                IDX[:, c, a, :],
                                m[:, jj:jj + 1].to_broadcast([128, 8]),
                                S[:, col: col + 80])

                # output assembly matmuls
                for c in range(nch):
                    mm1 = nc.tensor.matmul(
                        Pbx[:, (k * nch + c) * 18: (k * nch + c + 1) * 18],
                        FT[:, c * 128:(c + 1) * 128],
                        PB[(s, b)][:, :], start=True, stop=True)
                    _dep(mm1, *mm_deps)
                    mm2 = nc.tensor.matmul(
                        Pms[:, (k * nch + c) * 3: (k * nch + c + 1) * 3],
                        FT[:, c * 128:(c + 1) * 128],
                        PM[b][:, :], start=True, stop=True)
                    _dep(mm2, i_gt)

            nc.scalar.activation(Obox, Pbx,
                                 mybir.ActivationFunctionType.Copy)
            nc.scalar.activation(Omask, Pms,
                                 mybir.ActivationFunctionType.Copy)
            Obr = Obox.rearrange("p (c r) -> p c r", r=18)
            # cx/cy = (gx + x) * scale: the matmul produced the exact sum;
            # apply the scale here to match the reference rounding order
            Oxy = Obox.rearrange("p (c a f) -> p c a f", a=3, f=6)
            nc.vector.tensor_scalar_mul(Oxy[:, :, :, 1:3],
                                        Oxy[:, :, :, 1:3], scl128[s])
            for k, b in enumerate(grp):
                for a in range(3):
                    nc.vector.tensor_copy(
                        Obr[:, k * nch:(k + 1) * nch, 5 + 6 * a],
                        idx_of[b][:, :, a, 0])
            for k, b in enumerate(grp):
                _dma_out(nc, boxes_o, mask_o, Obox, Omask, i, hw, nch, b, k)


def _dma_out(nc, boxes_o, mask_o, Obox, Omask, i, hw, nch, b, k):
    base = LOCAL_BASES[i]
    nf = hw // 128
    cw = hw - nf * 128
    bo = boxes_o[:, :]
    mo = mask_o[:]
    row0 = base + b * hw * 3
    ob = Obox[:, k * nch * 18:]
    om = Omask[:, k * nch * 3:]
    if nf > 0:
        nc.sync.dma_start(
            out=_ap(bo, row0 * 6, [[18, 128], [2304, nf], [1, 18]]),
            in_=ob[:, 0: nf * 18])
        nc.sync.dma_start(
            out=_ap(mo, row0, [[3, 128], [384, nf], [1, 3]]),
            in_=om[:, 0: nf * 3])
    if cw > 0:
        nc.sync.dma_start(
            out=_ap(bo, (row0 + nf * 384) * 6, [[18, cw], [1, 18]]),
            in_=ob[0:cw, nf * 18: (nf + 1) * 18])
        nc.sync.dma_start(
            out=_ap(mo, row0 + nf * 384, [[3, cw], [1, 3]]),
            in_=om[0:cw, nf * 3: (nf + 1) * 3])


_PROGRAM = None
_LOCK = threading.Lock()


def _get_program():
    global _PROGRAM
    with _LOCK:
        if _PROGRAM is None:
            _PROGRAM = _build_program()
    return _PROGRAM


def _host_consts(anchors_13, anchors_26, anchors_52, thresh, case):
    case_f = float(np.asarray(case).reshape(-1)[0])
    anchors = {13: np.asarray(anchors_13, np.float32),
               26: np.asarray(anchors_26, np.float32),
               52: np.asarray(anchors_52, np.float32)}
    blob = np.zeros(NBLOB, np.float32)
    blob[0:12] = np.float32(np.asarray(thresh).reshape(-1)[0])
    goff = OFF_GXY
    for i, (s, t) in enumerate(SCALES):
        scale = np.float32(t / case_f)
        a = np.maximum(anchors[s].astype(np.float64) / case_f, 1e-38)
        la = np.log(a).astype(np.float32)
        blob[OFF_LNWH + i * 24: OFF_LNWH + i * 24 + 12] = np.repeat(la[:, 0], 4)
        blob[OFF_LNWH + i * 24 + 12: OFF_LNWH + i * 24 + 24] = (
            np.repeat(la[:, 1], 4))
        hwn = s * s
        idx = np.arange(hwn, dtype=np.float32)
        blob[goff: goff + hwn] = (idx % s).astype(np.float32)
        blob[goff + hwn: goff + 2 * hwn] = (
            np.floor(idx / s).astype(np.float32))
        goff += 2 * hwn
        blob[OFF_SCL + i * 128: OFF_SCL + (i + 1) * 128] = scale
        for b in range(B_LOCAL):
            P = np.zeros((128, 18), np.float32)
            for an in range(3):
                P[FROW[0] + 4 * an + b, an * 6 + 0] = 1.0
                P[FROW[1] + 4 * an + b, an * 6 + 1] = 1.0
                P[FROW[2] + 4 * an + b, an * 6 + 2] = 1.0
                P[FROW[3] + 4 * an + b, an * 6 + 3] = 1.0
                P[FROW[4] + 4 * an + b, an * 6 + 4] = 1.0
                P[GROW, an * 6 + 1] = 1.0
                P[GROW + 1, an * 6 + 2] = 1.0
            o = OFF_PB + (i * B_LOCAL + b) * 128 * 18
            blob[o: o + 128 * 18] = P.reshape(-1)
    for b in range(B_LOCAL):
        P = np.zeros((128, 3), np.float32)
        for an in range(3):
            P[MROW + 4 * an + b, an] = 1.0
        o = OFF_PM + b * 128 * 3
        blob[o: o + 128 * 3] = P.reshape(-1)
    ident = np.eye(128, dtype=np.float32)
    return blob, ident


def make_in_maps(output_13, output_26, output_52, anchors_13, anchors_26,
                 anchors_52, thresh, case):
    blob, ident = _host_consts(anchors_13, anchors_26, anchors_52,
                               thresh, case)
    outs = {13: np.asarray(output_13, np.float32),
            26: np.asarray(output_26, np.float32),
            52: np.asarray(output_52, np.float32)}
    in_maps = []
    for c in range(N_CORES):
        m = {"cblob": blob, "ident": ident}
        for s, _t in SCALES:
            hwn = s * s
            m[f"x{s}"] = np.ascontiguousarray(
                outs[s][c * B_LOCAL:(c + 1) * B_LOCAL].reshape(
                    B_LOCAL, 255, hwn))
        in_maps.append(m)
    return in_maps


def assemble(per_core_results):
    boxes = np.empty((N_FULL, 6), np.float32)
    mask = np.empty(N_FULL, np.float32)
    gbase = 0
    for i, hwn in enumerate(HWS):
        rows_per_b = hwn * 3
        n = B_LOCAL * rows_per_b
        lo = LOCAL_BASES[i]
        for c in range(N_CORES):
            gl = gbase + c * n
            boxes[gl:gl + n] = per_core_results[c]["boxes"][lo:lo + n]
            mask[gl:gl + n] = per_core_results[c]["maskf"][lo:lo + n]
        gbase += N_CORES * n
    return boxes, mask > 0.5


def kernel(output_13, output_26, output_52, anchors_13, anchors_26,
           anchors_52, thresh, case):
    nc = _get_program()
    in_maps = make_in_maps(output_13, output_26, output_52, anchors_13,
                           anchors_26, anchors_52, thresh, case)
    res = run_bass_kernel_spmd(nc, in_maps, core_ids=list(range(N_CORES)))
    return assemble(res.results)


# revision 37
# speedup vs baseline: 12385.3888x; 1.0060x over previous
"""YOLOv3-style detection decode kernel for Trainium2 (8 NeuronCores).

kernel(**inputs) takes the FULL unsharded inputs (as produced by
setup_inputs) and returns (boxes [N,6] f32, mask [N] bool) matching the
reference. The batch (32) is sharded 8 ways (4 per core); one SPMD
Bass/Tile program runs on all 8 cores with per-core input maps and the
host reassembles the full outputs.

Per-core pipeline (per scale):
  - contiguous channel-major loads: class slabs [80, HW] per (b, anchor);
    the five box-field channels into a single field tile FT [128, HWMAX]
    at aligned row bases (obj@0, x@32, y@44, w@64, h@76; +4a+b), with
    host-precomputed gx*scale / gy*scale rows at 120/121
  - batched field math: sigmoid(obj), exp(w/h + ln(anchor/case)),
    mask = obj > thresh (into rows 96..107)
  - output assembly as PE matmuls with per-(scale,b) constant
    permutation/affine matrices: out[cand, a*6+f] = sum_k FT[k,cand]*P[k,n]
    (x/y scaling and the grid add are baked into P; cls merged after)
  - exact argmax over the 80 classes: PE data-as-weights transposes
    [80,128] -> [128,80] into PSUM, segmented DVE reduce_max, ACT copy
    PSUM->SBUF, DVE max_index (first-occurrence, matching jnp.argmax)
  - contiguous DMA of boxes (18 f32 per candidate-chunk row) and mask
"""

import threading
from contextlib import ExitStack

import numpy as np

import concourse.bacc as bacc
import concourse.bass as bass
import concourse.mybir as mybir
import concourse.tile as tile
from concourse.bass_utils import run_bass_kernel_spmd
from concourse.tile import add_dep_helper


def _dep(frm, *tos):
    """Explicit dependency edges: raw (strided-partition) APs are tracked
    at last-writer granularity only, so readers depend on every writer
    explicitly."""
    f = getattr(frm, "ins", frm)
    for t in tos:
        add_dep_helper(f, getattr(t, "ins", t), reason="raw-ap-dep")


F32 = mybir.dt.float32
U32 = mybir.dt.uint32

SCALES = [(13, 32.0), (26, 16.0), (52, 8.0)]
B_LOCAL = 4
N_CORES = 8

HWS = [s * s for s, _ in SCALES]              # 169, 676, 2704
LOCAL_BASES = []
_acc = 0
for _hw in HWS:
    LOCAL_BASES.append(_acc)
    _acc += B_LOCAL * _hw * 3
N_LOCAL = _acc                                 # 42588
N_FULL = N_LOCAL * N_CORES                     # 340704

HWMAX = max(HWS)
CPADW_MAX = ((HWMAX + 127) // 128) * 128

# FT row bases: channel ch of (a, b) lands at FROW[ch] + 4a + b
FROW = [0, 32, 44, 64, 76]   # obj, x, y, w, h
MROW = 96                    # mask rows
ONES_ROW = 122               # constant 1.0 row (sigmoid = 0.5*tanh+0.5)
GROW = 120                   # gx*scale row; gy*scale at 121

# consts blob layout (element offsets)
OFF_LNWH = 12                         # per scale: 24 (ln aw x12, ln ah x12)
OFF_PB = 84                           # 12 x [128,18] (scale-major, b-minor)
OFF_PM = OFF_PB + 12 * 128 * 18       # 4 x [128,3]
OFF_GXY = OFF_PM + 4 * 128 * 3        # per scale: [2, hw]
OFF_SCL = OFF_GXY + 2 * sum(HWS)      # per scale: scale x128
NBLOB = OFF_SCL + 3 * 128

PAD_VAL = -1.0e30


def _ap(t, offset, ap):
    return bass.AP(tensor=t.tensor if isinstance(t, bass.AP) else t,
                   offset=offset, ap=ap)


def _build_program():
    nc = bacc.Bacc()
    xs = {}
    for (s, _t), hw in zip(SCALES, HWS):
        xs[s] = nc.dram_tensor(f"x{s}", [B_LOCAL, 255, hw], F32,
                               kind="ExternalInput")
    cblob = nc.dram_tensor("cblob", [NBLOB], F32, kind="ExternalInput")
    ident = nc.dram_tensor("ident", [128, 128], F32, kind="ExternalInput")
    boxes_o = nc.dram_tensor("boxes", [N_LOCAL, 6], F32, kind="ExternalOutput")
    mask_o = nc.dram_tensor("maskf", [N_LOCAL], F32, kind="ExternalOutput")

    import os
    prev = os.environ.get("BY_DEFAULT_DISABLE_SUBTILE_DEPS")
    os.environ["BY_DEFAULT_DISABLE_SUBTILE_DEPS"] = "1"
    try:
        with tile.TileContext(nc) as tc:
            with ExitStack() as ctx:
                _emit(ctx, tc, nc, xs, cblob, ident, boxes_o, mask_o)
    finally:
        if prev is None:
            os.environ.pop("BY_DEFAULT_DISABLE_SUBTILE_DEPS", None)
        else:
            os.environ["BY_DEFAULT_DISABLE_SUBTILE_DEPS"] = prev
    nc.compile()
    return nc


def _emit(ctx, tc, nc, xs, cblob, ident, boxes_o, mask_o):
    consts = ctx.enter_context(tc.tile_pool(name="consts", bufs=1))
    fpool = ctx.enter_context(tc.tile_pool(name="fpool", bufs=1))
    cpool = ctx.enter_context(tc.tile_pool(name="cpool", bufs=8))
    spool = ctx.enter_context(tc.tile_pool(name="spool", bufs=4))
    mpool = ctx.enter_context(tc.tile_pool(name="mpool", bufs=4))
    ipool = ctx.enter_context(tc.tile_pool(name="ipool", bufs=5))
    opool = ctx.enter_context(tc.tile_pool(name="opool", bufs=2))
    pc = ctx.enter_context(tc.tile_pool(name="pc", bufs=3, space="PSUM"))
    pob = ctx.enter_context(tc.tile_pool(name="pob", bufs=2, space="PSUM"))

    pr = cblob[:]
    id_sb = consts.tile([128, 128], F32, tag="id", name="id_sb")
    nc.sync.dma_start(out=id_sb, in_=ident[:, :])
    thresh_rep = consts.tile([12, 1], F32, tag="thresh", name="thresh_rep")
    nc.sync.dma_start(out=thresh_rep, in_=pr[0:12])

    # per-scale scale value replicated on all partitions (cx/cy scaling)
    scl128 = {}
    for i, (s, _t) in enumerate(SCALES):
        t = consts.tile([128, 1], F32, tag=f"sc{s}", name=f"sc{s}")
        nc.sync.dma_start(out=t,
                          in_=pr[OFF_SCL + i * 128: OFF_SCL + (i + 1) * 128])
        scl128[s] = t

    # exp bias tile: rows 64..87 <- [ln(aw/case) x12, ln(ah/case) x12]
    lnwh, d_lnwh = {}, {}
    for i, (s, _t) in enumerate(SCALES):
        lb = consts.tile([88, 1], F32, tag=f"ln{s}", name=f"ln{s}")
        d = nc.sync.dma_start(
            out=_ap(lb, 64, [[1, 24], [1, 1]]),
            in_=pr[OFF_LNWH + i * 24: OFF_LNWH + i * 24 + 24])
        lnwh[s], d_lnwh[s] = lb, d

    # permutation/affine matrices
    PB = {}
    for i, (s, _t) in enumerate(SCALES):
        for b in range(B_LOCAL):
            t = consts.tile([128, 18], F32, tag=f"PB{s}{b}",
                            name=f"PB{s}{b}")
            o = OFF_PB + (i * B_LOCAL + b) * 128 * 18
            nc.sync.dma_start(out=t, in_=pr[o: o + 128 * 18])
            PB[(s, b)] = t
    PM = {}
    for b in range(B_LOCAL):
        t = consts.tile([128, 3], F32, tag=f"PM{b}", name=f"PM{b}")
        o = OFF_PM + b * 128 * 3
        nc.sync.dma_start(out=t, in_=pr[o: o + 128 * 3])
        PM[b] = t

    # static field tile (memset once: unused rows must be exact 0.0 for the
    # assembly matmuls; used regions are overwritten every scale)
    FT = fpool.tile([128, CPADW_MAX], F32, tag="FT", name="FT")
    ft_ms = nc.gpsimd.memset(FT, 0.0)
    # compute-op partition bases must be 0/32/64/96: set the whole top
    # block to 1.0 (mask/grid rows are overwritten every scale; other
    # unused rows have zero P-matrix coefficients)
    ms_ones = nc.gpsimd.memset(FT[96:128, :], 1.0)
    _dep(ms_ones, ft_ms)

    goff = OFF_GXY
    for i, ((s, t), hw) in enumerate(zip(SCALES, HWS)):
        if i > 0:
            tc.strict_bb_all_engine_barrier()
        nch = (hw + 127) // 128
        cpadw = nch * 128
        x = xs[s][:, :, :]

        # --- loads ---------------------------------------------------------
        d_gxy = nc.sync.dma_start(
            out=_ap(FT, GROW * CPADW_MAX, [[CPADW_MAX, 2], [1, hw]]),
            in_=pr[goff: goff + 2 * hw])
        if i == 0:
            _dep(d_gxy, ft_ms)
        goff += 2 * hw

        fl = {}
        for ch in range(5):
            for a in range(3):
                d = nc.sync.dma_start(
                    out=_ap(FT, (FROW[ch] + 4 * a) * CPADW_MAX,
                            [[CPADW_MAX, 4], [1, hw]]),
                    in_=_ap(x, (a * 85 + ch) * hw, [[255 * hw, 4], [1, hw]]))
                if i == 0:
                    _dep(d, ft_ms)
                fl[(ch, a)] = d

        C = {}
        c_ms = {}
        for b in range(B_LOCAL):
            for a in range(3):
                Cb = cpool.tile([80, cpadw], F32, tag="C", name="Cb")
                C[(b, a)] = Cb
                if cpadw > hw:
                    c_ms[(b, a)] = nc.gpsimd.memset(Cb[:, hw:], PAD_VAL)
                nc.sync.dma_start(out=Cb[:, 0:hw],
                                  in_=x[b, a * 85 + 5: a * 85 + 85, :])

        # --- field math ----------------------------------------------------
        i_gt = nc.vector.tensor_scalar(FT[MROW:MROW + 12, 0:hw],
                                       FT[0:12, 0:hw], thresh_rep, None,
                                       op0=mybir.AluOpType.is_gt)
        _dep(i_gt, *[fl[(0, a)] for a in range(3)])
        i_sig = nc.scalar.activation(FT[0:12, 0:hw], FT[0:12, 0:hw],
                                     mybir.ActivationFunctionType.Tanh,
                                     scale=0.5)
        _dep(i_sig, i_gt, *[fl[(0, a)] for a in range(3)])
        i_exp = nc.scalar.activation(FT[64:88, 0:hw], FT[64:88, 0:hw],
                                     mybir.ActivationFunctionType.Exp,
                                     bias=lnwh[s][64:88])
        _dep(i_exp, d_lnwh[s],
             *[fl[(ch, a)] for ch in (3, 4) for a in range(3)])
        mm_deps = ([i_sig, i_exp, d_gxy, ms_ones]
                   + [fl[(ch, a)] for ch in (1, 2) for a in range(3)])

        # --- per-group assembly + argmax ----------------------------------
        groups = ([[b] for b in range(B_LOCAL)] if s == 52
                  else [list(range(B_LOCAL))])
        for grp in groups:
            ng = len(grp)
            # box cols [0 : ng*nch*18), mask cols after -- one PSUM bank
            Pbx = pob.tile([128, ng * nch * 21], F32, tag="pob", name="Pbx")
            moff = ng * nch * 18
            Obox = opool.tile([128, ng * nch * 18], F32, tag="Ob",
                              name="Obox")
            Omask = opool.tile([128, ng * nch * 3], F32, tag="Om",
                               name="Omask")
            idx_of = {}
            for k, b in enumerate(grp):
                IDX = ipool.tile([128, nch, 3, 8], U32, tag="IDX",
                                 name="IDX")
                idx_of[b] = IDX

                # class argmax: transposes packed 12 chunks / 2-bank tile
                for a in range(3):
                    Cb = C[(b, a)]
                    for g0 in range(0, nch, 12):
                        gn = min(12, nch - g0)
                        nb = (gn + 5) // 6
                        Pb = pc.tile([128, 1024], F32, tag="pc", name="Pb")
                        for jj in range(gn):
                            c = g0 + jj
                            col = (jj // 6) * 512 + (jj % 6) * 80
                            tr = nc.tensor.transpose(
                                Pb[:, col: col + 80],
                                Cb[:, c * 128:(c + 1) * 128],
                                id_sb[0:80, 0:80])
                            if c == nch - 1 and (b, a) in c_ms:
                                _dep(tr, c_ms[(b, a)])
                        m = mpool.tile([128, 12], F32, tag="m", name="m")
                        nbf, rem = gn // 6, gn % 6
                        if nbf:
                            Pr = Pb.rearrange(
                                "p (bk r) -> p bk r", bk=2)[
                                :, 0:nbf, 0:480].rearrange(
                                "p bk (g c) -> p bk g c", c=80)
                            nc.vector.tensor_reduce(
                                m[:, 0: nbf * 6], Pr,
                                axis=mybir.AxisListType.X,
                                op=mybir.AluOpType.max)
                        if rem:
                            Pr2 = Pb[:, nbf * 512: nbf * 512 + rem * 80
                                     ].rearrange("p (g c) -> p g c", c=80)
                            nc.vector.tensor_reduce(
                                m[:, nbf * 6: nbf * 6 + rem], Pr2,
                                axis=mybir.AxisListType.X,
                                op=mybir.AluOpType.max)
                        S = spool.tile([128, 1024], F32, tag="S", name="S")
                        for bk in range(nbf):
                            nc.scalar.activation(
                                S[:, bk * 512: bk * 512 + 480],
                                Pb[:, bk * 512: bk * 512 + 480],
                                mybir.ActivationFunctionType.Copy)
                        if rem:
                            nc.scalar.activation(
                                S[:, nbf * 512: nbf * 512 + rem * 80],
                                Pb[:, nbf * 512: nbf * 512 + rem * 80],
                                mybir.ActivationFunctionType.Copy)
                        for jj in range(gn):
                            c = g0 + jj
                            col = (jj // 6) * 512 + (jj % 6) * 80
                            nc.vector.max_index(
                                IDX[:, c, a, :],
                                m[:, jj:jj + 1].to_broadcast([128, 8]),
                                S[:, col: col + 80])

                # output assembly matmuls
                for c in range(nch):
                    mm1 = nc.tensor.matmul(
                        Pbx[:, (k * nch + c) * 18: (k * nch + c + 1) * 18],
                        FT[:, c * 128:(c + 1) * 128],
                        PB[(s, b)][:, :], start=True, stop=True)
                    _dep(mm1, *mm_deps)
                    mm2 = nc.tensor.matmul(
                        Pbx[:, moff + (k * nch + c) * 3:
                            moff + (k * nch + c + 1) * 3],
                        FT[:, c * 128:(c + 1) * 128],
                        PM[b][:, :], start=True, stop=True)
                    _dep(mm2, i_gt)

            nc.scalar.activation(Obox, Pbx[:, 0:moff],
                                 mybir.ActivationFunctionType.Copy)
            nc.scalar.activation(Omask, Pbx[:, moff:],
                                 mybir.ActivationFunctionType.Copy)
            Obr = Obox.rearrange("p (c r) -> p c r", r=18)
            # cx/cy = (gx + x) * scale: the matmul produced the exact sum;
            # apply the scale here to match the reference rounding order
            Oxy = Obox.rearrange("p (c a f) -> p c a f", a=3, f=6)
            nc.vector.tensor_scalar_mul(Oxy[:, :, :, 1:3],
                                        Oxy[:, :, :, 1:3], scl128[s])
            for k, b in enumerate(grp):
                for a in range(3):
                    nc.vector.tensor_copy(
                        Obr[:, k * nch:(k + 1) * nch, 5 + 6 * a],
                        idx_of[b][:, :, a, 0])
            for k, b in enumerate(grp):
                _dma_out(nc, boxes_o, mask_o, Obox, Omask, i, hw, nch, b, k)


def _dma_out(nc, boxes_o, mask_o, Obox, Omask, i, hw, nch, b, k):
    base = LOCAL_BASES[i]
    nf = hw // 128
    cw = hw - nf * 128
    bo = boxes_o[:, :]
    mo = mask_o[:]
    row0 = base + b * hw * 3
    ob = Obox[:, k * nch * 18:]
    om = Omask[:, k * nch * 3:]
    if nf > 0:
        nc.sync.dma_start(
            out=_ap(bo, row0 * 6, [[18, 128], [2304, nf], [1, 18]]),
            in_=ob[:, 0: nf * 18])
        nc.sync.dma_start(
            out=_ap(mo, row0, [[3, 128], [384, nf], [1, 3]]),
            in_=om[:, 0: nf * 3])
    if cw > 0:
        nc.sync.dma_start(
            out=_ap(bo, (row0 + nf * 384) * 6, [[18, cw], [1, 18]]),
            in_=ob[0:cw, nf * 18: (nf + 1) * 18])
        nc.sync.dma_start(
            out=_ap(mo, row0 + nf * 384, [[3, cw], [1, 3]]),
            in_=om[0:cw, nf * 3: (nf + 1) * 3])


_PROGRAM = None
_LOCK = threading.Lock()


def _get_program():
    global _PROGRAM
    with _LOCK:
        if _PROGRAM is None:
            _PROGRAM = _build_program()
    return _PROGRAM


def _host_consts(anchors_13, anchors_26, anchors_52, thresh, case):
    case_f = float(np.asarray(case).reshape(-1)[0])
    anchors = {13: np.asarray(anchors_13, np.float32),
               26: np.asarray(anchors_26, np.float32),
               52: np.asarray(anchors_52, np.float32)}
    blob = np.zeros(NBLOB, np.float32)
    blob[0:12] = np.float32(np.asarray(thresh).reshape(-1)[0])
    goff = OFF_GXY
    for i, (s, t) in enumerate(SCALES):
        scale = np.float32(t / case_f)
        a = np.maximum(anchors[s].astype(np.float64) / case_f, 1e-38)
        la = np.log(a).astype(np.float32)
        blob[OFF_LNWH + i * 24: OFF_LNWH + i * 24 + 12] = np.repeat(la[:, 0], 4)
        blob[OFF_LNWH + i * 24 + 12: OFF_LNWH + i * 24 + 24] = (
            np.repeat(la[:, 1], 4))
        hwn = s * s
        idx = np.arange(hwn, dtype=np.float32)
        blob[goff: goff + hwn] = (idx % s).astype(np.float32)
        blob[goff + hwn: goff + 2 * hwn] = (
            np.floor(idx / s).astype(np.float32))
        goff += 2 * hwn
        blob[OFF_SCL + i * 128: OFF_SCL + (i + 1) * 128] = scale
        for b in range(B_LOCAL):
            P = np.zeros((128, 18), np.float32)
            for an in range(3):
                P[FROW[0] + 4 * an + b, an * 6 + 0] = 0.5
                P[ONES_ROW, an * 6 + 0] = 0.5
                P[FROW[1] + 4 * an + b, an * 6 + 1] = 1.0
                P[FROW[2] + 4 * an + b, an * 6 + 2] = 1.0
                P[FROW[3] + 4 * an + b, an * 6 + 3] = 1.0
                P[FROW[4] + 4 * an + b, an * 6 + 4] = 1.0
                P[GROW, an * 6 + 1] = 1.0
                P[GROW + 1, an * 6 + 2] = 1.0
            o = OFF_PB + (i * B_LOCAL + b) * 128 * 18
            blob[o: o + 128 * 18] = P.reshape(-1)
    for b in range(B_LOCAL):
        P = np.zeros((128, 3), np.float32)
        for an in range(3):
            P[MROW + 4 * an + b, an] = 1.0
        o = OFF_PM + b * 128 * 3
        blob[o: o + 128 * 3] = P.reshape(-1)
    ident = np.eye(128, dtype=np.float32)
    return blob, ident


def make_in_maps(output_13, output_26, output_52, anchors_13, anchors_26,
                 anchors_52, thresh, case):
    blob, ident = _host_consts(anchors_13, anchors_26, anchors_52,
                               thresh, case)
    outs = {13: np.asarray(output_13, np.float32),
            26: np.asarray(output_26, np.float32),
            52: np.asarray(output_52, np.float32)}
    in_maps = []
    for c in range(N_CORES):
        m = {"cblob": blob, "ident": ident}
        for s, _t in SCALES:
            hwn = s * s
            m[f"x{s}"] = np.ascontiguousarray(
                outs[s][c * B_LOCAL:(c + 1) * B_LOCAL].reshape(
                    B_LOCAL, 255, hwn))
        in_maps.append(m)
    return in_maps


def assemble(per_core_results):
    boxes = np.empty((N_FULL, 6), np.float32)
    mask = np.empty(N_FULL, np.float32)
    gbase = 0
    for i, hwn in enumerate(HWS):
        rows_per_b = hwn * 3
        n = B_LOCAL * rows_per_b
        lo = LOCAL_BASES[i]
        for c in range(N_CORES):
            gl = gbase + c * n
            boxes[gl:gl + n] = per_core_results[c]["boxes"][lo:lo + n]
            mask[gl:gl + n] = per_core_results[c]["maskf"][lo:lo + n]
        gbase += N_CORES * n
    return boxes, mask > 0.5


def kernel(output_13, output_26, output_52, anchors_13, anchors_26,
           anchors_52, thresh, case):
    nc = _get_program()
    in_maps = make_in_maps(output_13, output_26, output_52, anchors_13,
                           anchors_26, anchors_52, thresh, case)
    res = run_bass_kernel_spmd(nc, in_maps, core_ids=list(range(N_CORES)))
    return assemble(res.results)


# revision 38
# speedup vs baseline: 12636.5354x; 1.0203x over previous
"""YOLOv3-style detection decode kernel for Trainium2 (8 NeuronCores).

kernel(**inputs) takes the FULL unsharded inputs (as produced by
setup_inputs) and returns (boxes [N,6] f32, mask [N] bool) matching the
reference. The batch (32) is sharded 8 ways (4 per core); one SPMD
Bass/Tile program runs on all 8 cores with per-core input maps and the
host reassembles the full outputs.

Per-core pipeline (per scale):
  - contiguous channel-major loads: class slabs [80, HW] per (b, anchor);
    the five box-field channels into a single field tile FT [128, HWMAX]
    at aligned row bases (obj@0, x@32, y@44, w@64, h@76; +4a+b), with
    host-precomputed gx*scale / gy*scale rows at 120/121
  - batched field math: sigmoid(obj), exp(w/h + ln(anchor/case)),
    mask = obj > thresh (into rows 96..107)
  - output assembly as PE matmuls with per-(scale,b) constant
    permutation/affine matrices: out[cand, a*6+f] = sum_k FT[k,cand]*P[k,n]
    (x/y scaling and the grid add are baked into P; cls merged after)
  - exact argmax over the 80 classes: PE data-as-weights transposes
    [80,128] -> [128,80] into PSUM, segmented DVE reduce_max, ACT copy
    PSUM->SBUF, DVE max_index (first-occurrence, matching jnp.argmax)
  - contiguous DMA of boxes (18 f32 per candidate-chunk row) and mask
"""

import threading
from contextlib import ExitStack

import numpy as np

import concourse.bacc as bacc
import concourse.bass as bass
import concourse.mybir as mybir
import concourse.tile as tile
from concourse.bass_utils import run_bass_kernel_spmd
from concourse.tile import add_dep_helper


def _dep(frm, *tos):
    """Explicit dependency edges: raw (strided-partition) APs are tracked
    at last-writer granularity only, so readers depend on every writer
    explicitly."""
    f = getattr(frm, "ins", frm)
    for t in tos:
        add_dep_helper(f, getattr(t, "ins", t), reason="raw-ap-dep")


F32 = mybir.dt.float32
U32 = mybir.dt.uint32

SCALES = [(13, 32.0), (26, 16.0), (52, 8.0)]
B_LOCAL = 4
N_CORES = 8

HWS = [s * s for s, _ in SCALES]              # 169, 676, 2704
LOCAL_BASES = []
_acc = 0
for _hw in HWS:
    LOCAL_BASES.append(_acc)
    _acc += B_LOCAL * _hw * 3
N_LOCAL = _acc                                 # 42588
N_FULL = N_LOCAL * N_CORES                     # 340704

HWMAX = max(HWS)
CPADW_MAX = ((HWMAX + 127) // 128) * 128

# FT row bases: channel ch of (a, b) lands at FROW[ch] + 4a + b
FROW = [0, 32, 44, 64, 76]   # obj, x, y, w, h
MROW = 96                    # mask rows
ONES_ROW = 122               # constant 1.0 row (sigmoid = 0.5*tanh+0.5)
GROW = 120                   # gx*scale row; gy*scale at 121

# consts blob layout (element offsets)
OFF_LNWH = 12                         # per scale: 24 (ln aw x12, ln ah x12)
OFF_PB = 84                           # 12 x [128,18] (scale-major, b-minor)
OFF_PM = OFF_PB + 12 * 128 * 18       # 4 x [128,3]
OFF_GXY = OFF_PM + 4 * 128 * 3        # per scale: [2, hw]
OFF_SCL = OFF_GXY + 2 * sum(HWS)      # per scale: scale x128
NBLOB = OFF_SCL + 3 * 128

PAD_VAL = -1.0e30


def _ap(t, offset, ap):
    return bass.AP(tensor=t.tensor if isinstance(t, bass.AP) else t,
                   offset=offset, ap=ap)


def _build_program():
    nc = bacc.Bacc()
    xs = {}
    for (s, _t), hw in zip(SCALES, HWS):
        xs[s] = nc.dram_tensor(f"x{s}", [B_LOCAL, 255, hw], F32,
                               kind="ExternalInput")
    cblob = nc.dram_tensor("cblob", [NBLOB], F32, kind="ExternalInput")
    ident = nc.dram_tensor("ident", [128, 128], F32, kind="ExternalInput")
    boxes_o = nc.dram_tensor("boxes", [N_LOCAL, 6], F32, kind="ExternalOutput")
    mask_o = nc.dram_tensor("maskf", [N_LOCAL], F32, kind="ExternalOutput")

    import os
    prev = os.environ.get("BY_DEFAULT_DISABLE_SUBTILE_DEPS")
    os.environ["BY_DEFAULT_DISABLE_SUBTILE_DEPS"] = "1"
    try:
        with tile.TileContext(nc) as tc:
            with ExitStack() as ctx:
                _emit(ctx, tc, nc, xs, cblob, ident, boxes_o, mask_o)
    finally:
        if prev is None:
            os.environ.pop("BY_DEFAULT_DISABLE_SUBTILE_DEPS", None)
        else:
            os.environ["BY_DEFAULT_DISABLE_SUBTILE_DEPS"] = prev
    nc.compile()
    return nc


def _emit(ctx, tc, nc, xs, cblob, ident, boxes_o, mask_o):
    consts = ctx.enter_context(tc.tile_pool(name="consts", bufs=1))
    fpool = ctx.enter_context(tc.tile_pool(name="fpool", bufs=1))
    cpool = ctx.enter_context(tc.tile_pool(name="cpool", bufs=8))
    spool = ctx.enter_context(tc.tile_pool(name="spool", bufs=4))
    mpool = ctx.enter_context(tc.tile_pool(name="mpool", bufs=4))
    ipool = ctx.enter_context(tc.tile_pool(name="ipool", bufs=5))
    opool = ctx.enter_context(tc.tile_pool(name="opool", bufs=2))
    pc = ctx.enter_context(tc.tile_pool(name="pc", bufs=3, space="PSUM"))
    pob = ctx.enter_context(tc.tile_pool(name="pob", bufs=2, space="PSUM"))

    pr = cblob[:]
    id_sb = consts.tile([128, 128], F32, tag="id", name="id_sb")
    nc.sync.dma_start(out=id_sb, in_=ident[:, :])
    thresh_rep = consts.tile([12, 1], F32, tag="thresh", name="thresh_rep")
    nc.sync.dma_start(out=thresh_rep, in_=pr[0:12])

    # per-scale scale value replicated on all partitions (cx/cy scaling)
    scl128 = {}
    for i, (s, _t) in enumerate(SCALES):
        t = consts.tile([128, 1], F32, tag=f"sc{s}", name=f"sc{s}")
        nc.sync.dma_start(out=t,
                          in_=pr[OFF_SCL + i * 128: OFF_SCL + (i + 1) * 128])
        scl128[s] = t

    # exp bias tile: rows 64..87 <- [ln(aw/case) x12, ln(ah/case) x12]
    lnwh, d_lnwh = {}, {}
    for i, (s, _t) in enumerate(SCALES):
        lb = consts.tile([88, 1], F32, tag=f"ln{s}", name=f"ln{s}")
        d = nc.sync.dma_start(
            out=_ap(lb, 64, [[1, 24], [1, 1]]),
            in_=pr[OFF_LNWH + i * 24: OFF_LNWH + i * 24 + 24])
        lnwh[s], d_lnwh[s] = lb, d

    # permutation/affine matrices
    PB = {}
    for i, (s, _t) in enumerate(SCALES):
        for b in range(B_LOCAL):
            t = consts.tile([128, 18], F32, tag=f"PB{s}{b}",
                            name=f"PB{s}{b}")
            o = OFF_PB + (i * B_LOCAL + b) * 128 * 18
            nc.sync.dma_start(out=t, in_=pr[o: o + 128 * 18])
            PB[(s, b)] = t
    PM = {}
    for b in range(B_LOCAL):
        t = consts.tile([128, 3], F32, tag=f"PM{b}", name=f"PM{b}")
        o = OFF_PM + b * 128 * 3
        nc.sync.dma_start(out=t, in_=pr[o: o + 128 * 3])
        PM[b] = t

    # static field tile (memset once: unused rows must be exact 0.0 for the
    # assembly matmuls; used regions are overwritten every scale)
    FT = fpool.tile([128, CPADW_MAX], F32, tag="FT", name="FT")
    ft_ms = nc.gpsimd.memset(FT, 0.0)
    # compute-op partition bases must be 0/32/64/96: set the whole top
    # block to 1.0 (mask/grid rows are overwritten every scale; other
    # unused rows have zero P-matrix coefficients)
    ms_ones = nc.gpsimd.memset(FT[96:128, :], 1.0)
    _dep(ms_ones, ft_ms)

    goff = OFF_GXY
    for i, ((s, t), hw) in enumerate(zip(SCALES, HWS)):
        if i > 0:
            tc.strict_bb_all_engine_barrier()
        nch = (hw + 127) // 128
        cpadw = nch * 128
        x = xs[s][:, :, :]

        # --- loads ---------------------------------------------------------
        d_gxy = nc.sync.dma_start(
            out=_ap(FT, GROW * CPADW_MAX, [[CPADW_MAX, 2], [1, hw]]),
            in_=pr[goff: goff + 2 * hw])
        if i == 0:
            _dep(d_gxy, ft_ms)
        goff += 2 * hw

        fl = {}
        for ch in range(5):
            for a in range(3):
                d = nc.sync.dma_start(
                    out=_ap(FT, (FROW[ch] + 4 * a) * CPADW_MAX,
                            [[CPADW_MAX, 4], [1, hw]]),
                    in_=_ap(x, (a * 85 + ch) * hw, [[255 * hw, 4], [1, hw]]))
                if i == 0:
                    _dep(d, ft_ms)
                fl[(ch, a)] = d

        C = {}
        c_ms = {}
        for b in range(B_LOCAL):
            for a in range(3):
                Cb = cpool.tile([80, cpadw], F32, tag="C", name="Cb")
                C[(b, a)] = Cb
                if cpadw > hw:
                    c_ms[(b, a)] = nc.gpsimd.memset(Cb[:, hw:], PAD_VAL)
                nc.sync.dma_start(out=Cb[:, 0:hw],
                                  in_=x[b, a * 85 + 5: a * 85 + 85, :])

        # --- field math ----------------------------------------------------
        i_gt = nc.gpsimd.tensor_scalar(FT[MROW:MROW + 12, 0:hw],
                                       FT[0:12, 0:hw], thresh_rep, None,
                                       op0=mybir.AluOpType.is_gt)
        _dep(i_gt, *[fl[(0, a)] for a in range(3)])
        i_sig = nc.scalar.activation(FT[0:12, 0:hw], FT[0:12, 0:hw],
                                     mybir.ActivationFunctionType.Tanh,
                                     scale=0.5)
        _dep(i_sig, i_gt, *[fl[(0, a)] for a in range(3)])
        i_exp = nc.scalar.activation(FT[64:88, 0:hw], FT[64:88, 0:hw],
                                     mybir.ActivationFunctionType.Exp,
                                     bias=lnwh[s][64:88])
        _dep(i_exp, d_lnwh[s],
             *[fl[(ch, a)] for ch in (3, 4) for a in range(3)])
        mm_deps = ([i_sig, i_exp, d_gxy, ms_ones]
                   + [fl[(ch, a)] for ch in (1, 2) for a in range(3)])

        # --- per-group assembly + argmax ----------------------------------
        groups = ([[b] for b in range(B_LOCAL)] if s == 52
                  else [list(range(B_LOCAL))])
        for grp in groups:
            ng = len(grp)
            # box cols [0 : ng*nch*18), mask cols after -- one PSUM bank
            Pbx = pob.tile([128, ng * nch * 21], F32, tag="pob", name="Pbx")
            moff = ng * nch * 18
            Obox = opool.tile([128, ng * nch * 18], F32, tag="Ob",
                              name="Obox")
            Omask = opool.tile([128, ng * nch * 3], F32, tag="Om",
                               name="Omask")
            idx_of = {}
            for k, b in enumerate(grp):
                IDX = ipool.tile([128, nch, 3, 8], U32, tag="IDX",
                                 name="IDX")
                idx_of[b] = IDX

                # class argmax: transposes packed 12 chunks / 2-bank tile
                for a in range(3):
                    Cb = C[(b, a)]
                    for g0 in range(0, nch, 12):
                        gn = min(12, nch - g0)
                        nb = (gn + 5) // 6
                        Pb = pc.tile([128, 1024], F32, tag="pc", name="Pb")
                        for jj in range(gn):
                            c = g0 + jj
                            col = (jj // 6) * 512 + (jj % 6) * 80
                            tr = nc.tensor.transpose(
                                Pb[:, col: col + 80],
                                Cb[:, c * 128:(c + 1) * 128],
                                id_sb[0:80, 0:80])
                            if c == nch - 1 and (b, a) in c_ms:
                                _dep(tr, c_ms[(b, a)])
                        m = mpool.tile([128, 12], F32, tag="m", name="m")
                        nbf, rem = gn // 6, gn % 6
                        if nbf:
                            Pr = Pb.rearrange(
                                "p (bk r) -> p bk r", bk=2)[
                                :, 0:nbf, 0:480].rearrange(
                                "p bk (g c) -> p bk g c", c=80)
                            nc.vector.tensor_reduce(
                                m[:, 0: nbf * 6], Pr,
                                axis=mybir.AxisListType.X,
                                op=mybir.AluOpType.max)
                        if rem:
                            Pr2 = Pb[:, nbf * 512: nbf * 512 + rem * 80
                                     ].rearrange("p (g c) -> p g c", c=80)
                            nc.vector.tensor_reduce(
                                m[:, nbf * 6: nbf * 6 + rem], Pr2,
                                axis=mybir.AxisListType.X,
                                op=mybir.AluOpType.max)
                        S = spool.tile([128, 1024], F32, tag="S", name="S")
                        for bk in range(nbf):
                            nc.scalar.activation(
                                S[:, bk * 512: bk * 512 + 480],
                                Pb[:, bk * 512: bk * 512 + 480],
                                mybir.ActivationFunctionType.Copy)
                        if rem:
                            nc.scalar.activation(
                                S[:, nbf * 512: nbf * 512 + rem * 80],
                                Pb[:, nbf * 512: nbf * 512 + rem * 80],
                                mybir.ActivationFunctionType.Copy)
                        for jj in range(gn):
                            c = g0 + jj
                            col = (jj // 6) * 512 + (jj % 6) * 80
                            nc.vector.max_index(
                                IDX[:, c, a, :],
                                m[:, jj:jj + 1].to_broadcast([128, 8]),
                                S[:, col: col + 80])

                # output assembly matmuls
                for c in range(nch):
                    mm1 = nc.tensor.matmul(
                        Pbx[:, (k * nch + c) * 18: (k * nch + c + 1) * 18],
                        FT[:, c * 128:(c + 1) * 128],
                        PB[(s, b)][:, :], start=True, stop=True)
                    _dep(mm1, *mm_deps)
                    mm2 = nc.tensor.matmul(
                        Pbx[:, moff + (k * nch + c) * 3:
                            moff + (k * nch + c + 1) * 3],
                        FT[:, c * 128:(c + 1) * 128],
                        PM[b][:, :], start=True, stop=True)
                    _dep(mm2, i_gt)

            nc.scalar.activation(Obox, Pbx[:, 0:moff],
                                 mybir.ActivationFunctionType.Copy)
            nc.scalar.activation(Omask, Pbx[:, moff:],
                                 mybir.ActivationFunctionType.Copy)
            Obr = Obox.rearrange("p (c r) -> p c r", r=18)
            # cx/cy = (gx + x) * scale: the matmul produced the exact sum;
            # apply the scale here to match the reference rounding order
            Oxy = Obox.rearrange("p (c a f) -> p c a f", a=3, f=6)
            nc.gpsimd.tensor_scalar_mul(Oxy[:, :, :, 1:3],
                                        Oxy[:, :, :, 1:3], scl128[s])
            for k, b in enumerate(grp):
                for a in range(3):
                    nc.vector.tensor_copy(
                        Obr[:, k * nch:(k + 1) * nch, 5 + 6 * a],
                        idx_of[b][:, :, a, 0])
            for k, b in enumerate(grp):
                _dma_out(nc, boxes_o, mask_o, Obox, Omask, i, hw, nch, b, k)


def _dma_out(nc, boxes_o, mask_o, Obox, Omask, i, hw, nch, b, k):
    base = LOCAL_BASES[i]
    nf = hw // 128
    cw = hw - nf * 128
    bo = boxes_o[:, :]
    mo = mask_o[:]
    row0 = base + b * hw * 3
    ob = Obox[:, k * nch * 18:]
    om = Omask[:, k * nch * 3:]
    if nf > 0:
        nc.sync.dma_start(
            out=_ap(bo, row0 * 6, [[18, 128], [2304, nf], [1, 18]]),
            in_=ob[:, 0: nf * 18])
        nc.sync.dma_start(
            out=_ap(mo, row0, [[3, 128], [384, nf], [1, 3]]),
            in_=om[:, 0: nf * 3])
    if cw > 0:
        nc.sync.dma_start(
            out=_ap(bo, (row0 + nf * 384) * 6, [[18, cw], [1, 18]]),
            in_=ob[0:cw, nf * 18: (nf + 1) * 18])
        nc.sync.dma_start(
            out=_ap(mo, row0 + nf * 384, [[3, cw], [1, 3]]),
            in_=om[0:cw, nf * 3: (nf + 1) * 3])


_PROGRAM = None
_LOCK = threading.Lock()


def _get_program():
    global _PROGRAM
    with _LOCK:
        if _PROGRAM is None:
            _PROGRAM = _build_program()
    return _PROGRAM


def _host_consts(anchors_13, anchors_26, anchors_52, thresh, case):
    case_f = float(np.asarray(case).reshape(-1)[0])
    anchors = {13: np.asarray(anchors_13, np.float32),
               26: np.asarray(anchors_26, np.float32),
               52: np.asarray(anchors_52, np.float32)}
    blob = np.zeros(NBLOB, np.float32)
    blob[0:12] = np.float32(np.asarray(thresh).reshape(-1)[0])
    goff = OFF_GXY
    for i, (s, t) in enumerate(SCALES):
        scale = np.float32(t / case_f)
        a = np.maximum(anchors[s].astype(np.float64) / case_f, 1e-38)
        la = np.log(a).astype(np.float32)
        blob[OFF_LNWH + i * 24: OFF_LNWH + i * 24 + 12] = np.repeat(la[:, 0], 4)
        blob[OFF_LNWH + i * 24 + 12: OFF_LNWH + i * 24 + 24] = (
            np.repeat(la[:, 1], 4))
        hwn = s * s
        idx = np.arange(hwn, dtype=np.float32)
        blob[goff: goff + hwn] = (idx % s).astype(np.float32)
        blob[goff + hwn: goff + 2 * hwn] = (
            np.floor(idx / s).astype(np.float32))
        goff += 2 * hwn
        blob[OFF_SCL + i * 128: OFF_SCL + (i + 1) * 128] = scale
        for b in range(B_LOCAL):
            P = np.zeros((128, 18), np.float32)
            for an in range(3):
                P[FROW[0] + 4 * an + b, an * 6 + 0] = 0.5
                P[ONES_ROW, an * 6 + 0] = 0.5
                P[FROW[1] + 4 * an + b, an * 6 + 1] = 1.0
                P[FROW[2] + 4 * an + b, an * 6 + 2] = 1.0
                P[FROW[3] + 4 * an + b, an * 6 + 3] = 1.0
                P[FROW[4] + 4 * an + b, an * 6 + 4] = 1.0
                P[GROW, an * 6 + 1] = 1.0
                P[GROW + 1, an * 6 + 2] = 1.0
            o = OFF_PB + (i * B_LOCAL + b) * 128 * 18
            blob[o: o + 128 * 18] = P.reshape(-1)
    for b in range(B_LOCAL):
        P = np.zeros((128, 3), np.float32)
        for an in range(3):
            P[MROW + 4 * an + b, an] = 1.0
        o = OFF_PM + b * 128 * 3
        blob[o: o + 128 * 3] = P.reshape(-1)
    ident = np.eye(128, dtype=np.float32)
    return blob, ident


def make_in_maps(output_13, output_26, output_52, anchors_13, anchors_26,
                 anchors_52, thresh, case):
    blob, ident = _host_consts(anchors_13, anchors_26, anchors_52,
                               thresh, case)
    outs = {13: np.asarray(output_13, np.float32),
            26: np.asarray(output_26, np.float32),
            52: np.asarray(output_52, np.float32)}
    in_maps = []
    for c in range(N_CORES):
        m = {"cblob": blob, "ident": ident}
        for s, _t in SCALES:
            hwn = s * s
            m[f"x{s}"] = np.ascontiguousarray(
                outs[s][c * B_LOCAL:(c + 1) * B_LOCAL].reshape(
                    B_LOCAL, 255, hwn))
        in_maps.append(m)
    return in_maps


def assemble(per_core_results):
    boxes = np.empty((N_FULL, 6), np.float32)
    mask = np.empty(N_FULL, np.float32)
    gbase = 0
    for i, hwn in enumerate(HWS):
        rows_per_b = hwn * 3
        n = B_LOCAL * rows_per_b
        lo = LOCAL_BASES[i]
        for c in range(N_CORES):
            gl = gbase + c * n
            boxes[gl:gl + n] = per_core_results[c]["boxes"][lo:lo + n]
            mask[gl:gl + n] = per_core_results[c]["maskf"][lo:lo + n]
        gbase += N_CORES * n
    return boxes, mask > 0.5


def kernel(output_13, output_26, output_52, anchors_13, anchors_26,
           anchors_52, thresh, case):
    nc = _get_program()
    in_maps = make_in_maps(output_13, output_26, output_52, anchors_13,
                           anchors_26, anchors_52, thresh, case)
    res = run_bass_kernel_spmd(nc, in_maps, core_ids=list(range(N_CORES)))
    return assemble(res.results)


# revision 39
# speedup vs baseline: 13485.5593x; 1.0672x over previous
"""YOLOv3-style detection decode kernel for Trainium2 (8 NeuronCores).

kernel(**inputs) takes the FULL unsharded inputs (as produced by
setup_inputs) and returns (boxes [N,6] f32, mask [N] bool) matching the
reference. The batch (32) is sharded 8 ways (4 per core); one SPMD
Bass/Tile program runs on all 8 cores with per-core input maps and the
host reassembles the full outputs.

Per-core pipeline (per scale):
  - contiguous channel-major loads: class slabs [80, HW] per (b, anchor);
    the five box-field channels into a single field tile FT [128, HWMAX]
    at aligned row bases (obj@0, x@32, y@44, w@64, h@76; +4a+b), with
    host-precomputed gx*scale / gy*scale rows at 120/121
  - batched field math: sigmoid(obj), exp(w/h + ln(anchor/case)),
    mask = obj > thresh (into rows 96..107)
  - output assembly as PE matmuls with per-(scale,b) constant
    permutation/affine matrices: out[cand, a*6+f] = sum_k FT[k,cand]*P[k,n]
    (x/y scaling and the grid add are baked into P; cls merged after)
  - exact argmax over the 80 classes: PE data-as-weights transposes
    [80,128] -> [128,80] into PSUM, segmented DVE reduce_max, ACT copy
    PSUM->SBUF, DVE max_index (first-occurrence, matching jnp.argmax)
  - contiguous DMA of boxes (18 f32 per candidate-chunk row) and mask
"""

import threading
from contextlib import ExitStack

import numpy as np

import concourse.bacc as bacc
import concourse.bass as bass
import concourse.mybir as mybir
import concourse.tile as tile
from concourse.bass_utils import run_bass_kernel_spmd
from concourse.tile import add_dep_helper


def _dep(frm, *tos):
    """Explicit dependency edges: raw (strided-partition) APs are tracked
    at last-writer granularity only, so readers depend on every writer
    explicitly."""
    f = getattr(frm, "ins", frm)
    for t in tos:
        add_dep_helper(f, getattr(t, "ins", t), reason="raw-ap-dep")


F32 = mybir.dt.float32
U32 = mybir.dt.uint32

SCALES = [(13, 32.0), (26, 16.0), (52, 8.0)]
B_LOCAL = 4
N_CORES = 8

HWS = [s * s for s, _ in SCALES]              # 169, 676, 2704
LOCAL_BASES = []
_acc = 0
for _hw in HWS:
    LOCAL_BASES.append(_acc)
    _acc += B_LOCAL * _hw * 3
N_LOCAL = _acc                                 # 42588
N_FULL = N_LOCAL * N_CORES                     # 340704

HWMAX = max(HWS)
CPADW_MAX = ((HWMAX + 127) // 128) * 128

# FT row bases: channel ch of (a, b) lands at FROW[ch] + 4a + b
FROW = [0, 32, 44, 64, 76]   # obj, x, y, w, h
MROW = 96                    # mask rows
ONES_ROW = 122               # constant 1.0 row (sigmoid = 0.5*tanh+0.5)
GROW = 120                   # gx*scale row; gy*scale at 121

# consts blob layout (element offsets)
OFF_LNWH = 12                         # per scale: 24 (ln aw x12, ln ah x12)
OFF_PB = 84                           # 12 x [128,18] (scale-major, b-minor)
OFF_PM = OFF_PB + 12 * 128 * 18       # 4 x [128,3]
OFF_GXY = OFF_PM + 4 * 128 * 3        # per scale: [2, hw]
OFF_SCL = OFF_GXY + 2 * sum(HWS)      # per scale: scale x128
NBLOB = OFF_SCL + 3 * 128

PAD_VAL = -1.0e30


def _ap(t, offset, ap):
    return bass.AP(tensor=t.tensor if isinstance(t, bass.AP) else t,
                   offset=offset, ap=ap)


def _build_program():
    nc = bacc.Bacc()
    xs = {}
    for (s, _t), hw in zip(SCALES, HWS):
        xs[s] = nc.dram_tensor(f"x{s}", [B_LOCAL, 255, hw], F32,
                               kind="ExternalInput")
    cblob = nc.dram_tensor("cblob", [NBLOB], F32, kind="ExternalInput")
    ident = nc.dram_tensor("ident", [128, 128], F32, kind="ExternalInput")
    boxes_o = nc.dram_tensor("boxes", [N_LOCAL, 6], F32, kind="ExternalOutput")
    mask_o = nc.dram_tensor("maskf", [N_LOCAL], F32, kind="ExternalOutput")

    import os
    prev = os.environ.get("BY_DEFAULT_DISABLE_SUBTILE_DEPS")
    os.environ["BY_DEFAULT_DISABLE_SUBTILE_DEPS"] = "1"
    try:
        with tile.TileContext(nc) as tc:
            with ExitStack() as ctx:
                _emit(ctx, tc, nc, xs, cblob, ident, boxes_o, mask_o)
    finally:
        if prev is None:
            os.environ.pop("BY_DEFAULT_DISABLE_SUBTILE_DEPS", None)
        else:
            os.environ["BY_DEFAULT_DISABLE_SUBTILE_DEPS"] = prev
    nc.compile()
    return nc


def _emit(ctx, tc, nc, xs, cblob, ident, boxes_o, mask_o):
    consts = ctx.enter_context(tc.tile_pool(name="consts", bufs=1))
    fpool = ctx.enter_context(tc.tile_pool(name="fpool", bufs=1))
    cpool = ctx.enter_context(tc.tile_pool(name="cpool", bufs=8))
    spool = ctx.enter_context(tc.tile_pool(name="spool", bufs=5))
    mpool = ctx.enter_context(tc.tile_pool(name="mpool", bufs=4))
    ipool = ctx.enter_context(tc.tile_pool(name="ipool", bufs=5))
    opool = ctx.enter_context(tc.tile_pool(name="opool", bufs=2))
    pc = ctx.enter_context(tc.tile_pool(name="pc", bufs=3, space="PSUM"))
    pob = ctx.enter_context(tc.tile_pool(name="pob", bufs=2, space="PSUM"))

    pr = cblob[:]
    id_sb = consts.tile([128, 128], F32, tag="id", name="id_sb")
    nc.sync.dma_start(out=id_sb, in_=ident[:, :])
    thresh_rep = consts.tile([12, 1], F32, tag="thresh", name="thresh_rep")
    nc.sync.dma_start(out=thresh_rep, in_=pr[0:12])

    # per-scale scale value replicated on all partitions (cx/cy scaling)
    scl128 = {}
    for i, (s, _t) in enumerate(SCALES):
        t = consts.tile([128, 1], F32, tag=f"sc{s}", name=f"sc{s}")
        nc.sync.dma_start(out=t,
                          in_=pr[OFF_SCL + i * 128: OFF_SCL + (i + 1) * 128])
        scl128[s] = t

    # exp bias tile: rows 64..87 <- [ln(aw/case) x12, ln(ah/case) x12]
    lnwh, d_lnwh = {}, {}
    for i, (s, _t) in enumerate(SCALES):
        lb = consts.tile([88, 1], F32, tag=f"ln{s}", name=f"ln{s}")
        d = nc.sync.dma_start(
            out=_ap(lb, 64, [[1, 24], [1, 1]]),
            in_=pr[OFF_LNWH + i * 24: OFF_LNWH + i * 24 + 24])
        lnwh[s], d_lnwh[s] = lb, d

    # permutation/affine matrices
    PB = {}
    for i, (s, _t) in enumerate(SCALES):
        for b in range(B_LOCAL):
            t = consts.tile([128, 18], F32, tag=f"PB{s}{b}",
                            name=f"PB{s}{b}")
            o = OFF_PB + (i * B_LOCAL + b) * 128 * 18
            nc.sync.dma_start(out=t, in_=pr[o: o + 128 * 18])
            PB[(s, b)] = t
    PM = {}
    for b in range(B_LOCAL):
        t = consts.tile([128, 3], F32, tag=f"PM{b}", name=f"PM{b}")
        o = OFF_PM + b * 128 * 3
        nc.sync.dma_start(out=t, in_=pr[o: o + 128 * 3])
        PM[b] = t

    # static field tile (memset once: unused rows must be exact 0.0 for the
    # assembly matmuls; used regions are overwritten every scale)
    FT = fpool.tile([128, CPADW_MAX], F32, tag="FT", name="FT")
    ft_ms = nc.gpsimd.memset(FT, 0.0)
    # compute-op partition bases must be 0/32/64/96: set the whole top
    # block to 1.0 (mask/grid rows are overwritten every scale; other
    # unused rows have zero P-matrix coefficients)
    ms_ones = nc.gpsimd.memset(FT[96:128, :], 1.0)
    _dep(ms_ones, ft_ms)

    goff = OFF_GXY
    for i, ((s, t), hw) in enumerate(zip(SCALES, HWS)):
        if i > 0:
            tc.strict_bb_all_engine_barrier()
        nch = (hw + 127) // 128
        cpadw = nch * 128
        x = xs[s][:, :, :]

        # --- loads ---------------------------------------------------------
        d_gxy = nc.sync.dma_start(
            out=_ap(FT, GROW * CPADW_MAX, [[CPADW_MAX, 2], [1, hw]]),
            in_=pr[goff: goff + 2 * hw])
        if i == 0:
            _dep(d_gxy, ft_ms)
        goff += 2 * hw

        fl = {}
        for ch in range(5):
            for a in range(3):
                d = nc.sync.dma_start(
                    out=_ap(FT, (FROW[ch] + 4 * a) * CPADW_MAX,
                            [[CPADW_MAX, 4], [1, hw]]),
                    in_=_ap(x, (a * 85 + ch) * hw, [[255 * hw, 4], [1, hw]]))
                if i == 0:
                    _dep(d, ft_ms)
                fl[(ch, a)] = d

        C = {}
        c_ms = {}
        for b in range(B_LOCAL):
            for a in range(3):
                Cb = cpool.tile([80, cpadw], F32, tag="C", name="Cb")
                C[(b, a)] = Cb
                if cpadw > hw:
                    c_ms[(b, a)] = nc.gpsimd.memset(Cb[:, hw:], PAD_VAL)
                nc.sync.dma_start(out=Cb[:, 0:hw],
                                  in_=x[b, a * 85 + 5: a * 85 + 85, :])

        # --- field math ----------------------------------------------------
        i_gt = nc.gpsimd.tensor_scalar(FT[MROW:MROW + 12, 0:hw],
                                       FT[0:12, 0:hw], thresh_rep, None,
                                       op0=mybir.AluOpType.is_gt)
        _dep(i_gt, *[fl[(0, a)] for a in range(3)])
        i_sig = nc.scalar.activation(FT[0:12, 0:hw], FT[0:12, 0:hw],
                                     mybir.ActivationFunctionType.Tanh,
                                     scale=0.5)
        _dep(i_sig, i_gt, *[fl[(0, a)] for a in range(3)])
        i_exp = nc.scalar.activation(FT[64:88, 0:hw], FT[64:88, 0:hw],
                                     mybir.ActivationFunctionType.Exp,
                                     bias=lnwh[s][64:88])
        _dep(i_exp, d_lnwh[s],
             *[fl[(ch, a)] for ch in (3, 4) for a in range(3)])
        mm_deps = ([i_sig, i_exp, d_gxy, ms_ones]
                   + [fl[(ch, a)] for ch in (1, 2) for a in range(3)])

        # --- per-group assembly + argmax ----------------------------------
        groups = ([[b] for b in range(B_LOCAL)] if s == 52
                  else [list(range(B_LOCAL))])
        for grp in groups:
            ng = len(grp)
            # box cols [0 : ng*nch*18), mask cols after -- one PSUM bank
            Pbx = pob.tile([128, ng * nch * 21], F32, tag="pob", name="Pbx")
            moff = ng * nch * 18
            Obox = opool.tile([128, ng * nch * 18], F32, tag="Ob",
                              name="Obox")
            Omask = opool.tile([128, ng * nch * 3], F32, tag="Om",
                               name="Omask")
            idx_of = {}
            for k, b in enumerate(grp):
                IDX = ipool.tile([128, nch, 3, 8], U32, tag="IDX",
                                 name="IDX")
                idx_of[b] = IDX

                # class argmax: transposes packed 12 chunks / 2-bank tile
                for a in range(3):
                    Cb = C[(b, a)]
                    for g0 in range(0, nch, 12):
                        gn = min(12, nch - g0)
                        nb = (gn + 5) // 6
                        Pb = pc.tile([128, 1024], F32, tag="pc", name="Pb")
                        for jj in range(gn):
                            c = g0 + jj
                            col = (jj // 6) * 512 + (jj % 6) * 80
                            tr = nc.tensor.transpose(
                                Pb[:, col: col + 80],
                                Cb[:, c * 128:(c + 1) * 128],
                                id_sb[0:80, 0:80])
                            if c == nch - 1 and (b, a) in c_ms:
                                _dep(tr, c_ms[(b, a)])
                        m = mpool.tile([128, 12], F32, tag="m", name="m")
                        nbf, rem = gn // 6, gn % 6
                        S = spool.tile([128, 1024], F32, tag="S", name="S")
                        for bk in range(nbf):
                            nc.scalar.activation(
                                S[:, bk * 512: bk * 512 + 480],
                                Pb[:, bk * 512: bk * 512 + 480],
                                mybir.ActivationFunctionType.Copy)
                        if rem:
                            nc.scalar.activation(
                                S[:, nbf * 512: nbf * 512 + rem * 80],
                                Pb[:, nbf * 512: nbf * 512 + rem * 80],
                                mybir.ActivationFunctionType.Copy)
                        # reduce from SBUF (cheaper init, frees the PSUM
                        # bank as soon as the copy is done)
                        if nbf:
                            Sr = S.rearrange(
                                "p (bk r) -> p bk r", bk=2)[
                                :, 0:nbf, 0:480].rearrange(
                                "p bk (g c) -> p bk g c", c=80)
                            nc.vector.tensor_reduce(
                                m[:, 0: nbf * 6], Sr,
                                axis=mybir.AxisListType.X,
                                op=mybir.AluOpType.max)
                        if rem:
                            Sr2 = S[:, nbf * 512: nbf * 512 + rem * 80
                                    ].rearrange("p (g c) -> p g c", c=80)
                            nc.vector.tensor_reduce(
                                m[:, nbf * 6: nbf * 6 + rem], Sr2,
                                axis=mybir.AxisListType.X,
                                op=mybir.AluOpType.max)
                        for jj in range(gn):
                            c = g0 + jj
                            col = (jj // 6) * 512 + (jj % 6) * 80
                            nc.vector.max_index(
                                IDX[:, c, a, :],
                                m[:, jj:jj + 1].to_broadcast([128, 8]),
                                S[:, col: col + 80])

                # output assembly matmuls
                for c in range(nch):
                    mm1 = nc.tensor.matmul(
                        Pbx[:, (k * nch + c) * 18: (k * nch + c + 1) * 18],
                        FT[:, c * 128:(c + 1) * 128],
                        PB[(s, b)][:, :], start=True, stop=True)
                    _dep(mm1, *mm_deps)
                    mm2 = nc.tensor.matmul(
                        Pbx[:, moff + (k * nch + c) * 3:
                            moff + (k * nch + c + 1) * 3],
                        FT[:, c * 128:(c + 1) * 128],
                        PM[b][:, :], start=True, stop=True)
                    _dep(mm2, i_gt)

            nc.scalar.activation(Obox, Pbx[:, 0:moff],
                                 mybir.ActivationFunctionType.Copy)
            nc.scalar.activation(Omask, Pbx[:, moff:],
                                 mybir.ActivationFunctionType.Copy)
            Obr = Obox.rearrange("p (c r) -> p c r", r=18)
            # cx/cy = (gx + x) * scale: the matmul produced the exact sum;
            # apply the scale here to match the reference rounding order
            Oxy = Obox.rearrange("p (c a f) -> p c a f", a=3, f=6)
            nc.gpsimd.tensor_scalar_mul(Oxy[:, :, :, 1:3],
                                        Oxy[:, :, :, 1:3], scl128[s])
            for k, b in enumerate(grp):
                for a in range(3):
                    nc.vector.tensor_copy(
                        Obr[:, k * nch:(k + 1) * nch, 5 + 6 * a],
                        idx_of[b][:, :, a, 0])
            for k, b in enumerate(grp):
                _dma_out(nc, boxes_o, mask_o, Obox, Omask, i, hw, nch, b, k)


def _dma_out(nc, boxes_o, mask_o, Obox, Omask, i, hw, nch, b, k):
    base = LOCAL_BASES[i]
    nf = hw // 128
    cw = hw - nf * 128
    bo = boxes_o[:, :]
    mo = mask_o[:]
    row0 = base + b * hw * 3
    ob = Obox[:, k * nch * 18:]
    om = Omask[:, k * nch * 3:]
    if nf > 0:
        nc.sync.dma_start(
            out=_ap(bo, row0 * 6, [[18, 128], [2304, nf], [1, 18]]),
            in_=ob[:, 0: nf * 18])
        nc.sync.dma_start(
            out=_ap(mo, row0, [[3, 128], [384, nf], [1, 3]]),
            in_=om[:, 0: nf * 3])
    if cw > 0:
        nc.sync.dma_start(
            out=_ap(bo, (row0 + nf * 384) * 6, [[18, cw], [1, 18]]),
            in_=ob[0:cw, nf * 18: (nf + 1) * 18])
        nc.sync.dma_start(
            out=_ap(mo, row0 + nf * 384, [[3, cw], [1, 3]]),
            in_=om[0:cw, nf * 3: (nf + 1) * 3])


_PROGRAM = None
_LOCK = threading.Lock()


def _get_program():
    global _PROGRAM
    with _LOCK:
        if _PROGRAM is None:
            _PROGRAM = _build_program()
    return _PROGRAM


def _host_consts(anchors_13, anchors_26, anchors_52, thresh, case):
    case_f = float(np.asarray(case).reshape(-1)[0])
    anchors = {13: np.asarray(anchors_13, np.float32),
               26: np.asarray(anchors_26, np.float32),
               52: np.asarray(anchors_52, np.float32)}
    blob = np.zeros(NBLOB, np.float32)
    blob[0:12] = np.float32(np.asarray(thresh).reshape(-1)[0])
    goff = OFF_GXY
    for i, (s, t) in enumerate(SCALES):
        scale = np.float32(t / case_f)
        a = np.maximum(anchors[s].astype(np.float64) / case_f, 1e-38)
        la = np.log(a).astype(np.float32)
        blob[OFF_LNWH + i * 24: OFF_LNWH + i * 24 + 12] = np.repeat(la[:, 0], 4)
        blob[OFF_LNWH + i * 24 + 12: OFF_LNWH + i * 24 + 24] = (
            np.repeat(la[:, 1], 4))
        hwn = s * s
        idx = np.arange(hwn, dtype=np.float32)
        blob[goff: goff + hwn] = (idx % s).astype(np.float32)
        blob[goff + hwn: goff + 2 * hwn] = (
            np.floor(idx / s).astype(np.float32))
        goff += 2 * hwn
        blob[OFF_SCL + i * 128: OFF_SCL + (i + 1) * 128] = scale
        for b in range(B_LOCAL):
            P = np.zeros((128, 18), np.float32)
            for an in range(3):
                P[FROW[0] + 4 * an + b, an * 6 + 0] = 0.5
                P[ONES_ROW, an * 6 + 0] = 0.5
                P[FROW[1] + 4 * an + b, an * 6 + 1] = 1.0
                P[FROW[2] + 4 * an + b, an * 6 + 2] = 1.0
                P[FROW[3] + 4 * an + b, an * 6 + 3] = 1.0
                P[FROW[4] + 4 * an + b, an * 6 + 4] = 1.0
                P[GROW, an * 6 + 1] = 1.0
                P[GROW + 1, an * 6 + 2] = 1.0
            o = OFF_PB + (i * B_LOCAL + b) * 128 * 18
            blob[o: o + 128 * 18] = P.reshape(-1)
    for b in range(B_LOCAL):
        P = np.zeros((128, 3), np.float32)
        for an in range(3):
            P[MROW + 4 * an + b, an] = 1.0
        o = OFF_PM + b * 128 * 3
        blob[o: o + 128 * 3] = P.reshape(-1)
    ident = np.eye(128, dtype=np.float32)
    return blob, ident


def make_in_maps(output_13, output_26, output_52, anchors_13, anchors_26,
                 anchors_52, thresh, case):
    blob, ident = _host_consts(anchors_13, anchors_26, anchors_52,
                               thresh, case)
    outs = {13: np.asarray(output_13, np.float32),
            26: np.asarray(output_26, np.float32),
            52: np.asarray(output_52, np.float32)}
    in_maps = []
    for c in range(N_CORES):
        m = {"cblob": blob, "ident": ident}
        for s, _t in SCALES:
            hwn = s * s
            m[f"x{s}"] = np.ascontiguousarray(
                outs[s][c * B_LOCAL:(c + 1) * B_LOCAL].reshape(
                    B_LOCAL, 255, hwn))
        in_maps.append(m)
    return in_maps


def assemble(per_core_results):
    boxes = np.empty((N_FULL, 6), np.float32)
    mask = np.empty(N_FULL, np.float32)
    gbase = 0
    for i, hwn in enumerate(HWS):
        rows_per_b = hwn * 3
        n = B_LOCAL * rows_per_b
        lo = LOCAL_BASES[i]
        for c in range(N_CORES):
            gl = gbase + c * n
            boxes[gl:gl + n] = per_core_results[c]["boxes"][lo:lo + n]
            mask[gl:gl + n] = per_core_results[c]["maskf"][lo:lo + n]
        gbase += N_CORES * n
    return boxes, mask > 0.5


def kernel(output_13, output_26, output_52, anchors_13, anchors_26,
           anchors_52, thresh, case):
    nc = _get_program()
    in_maps = make_in_maps(output_13, output_26, output_52, anchors_13,
                           anchors_26, anchors_52, thresh, case)
    res = run_bass_kernel_spmd(nc, in_maps, core_ids=list(range(N_CORES)))
    return assemble(res.results)
